# revision 12
# baseline (speedup 1.0000x reference)
"""AttentionDecoderRNN single-step decode on 8 TRN2 NeuronCores.

Strategy (tensor-parallel matvec, 2 AllGathers total):
  - attn_W is sharded over output rows: core i computes logits for
    l in [i*512, (i+1)*512).  encoder_outputs is sharded over the same
    rows, so core i can compute an *unnormalized* attention-value
    partial u_i = sum_j exp(logit_ij - m_i) * enc_i[j, :] plus the
    local stats (m_i, s_i) -- flash-attention style.
  - AllGather #1 carries {u_i (1024), s_i, m_i, logits_i (512)} per
    rank.  Every core then reconstructs the exact global softmax via
    rescaling coefficients c_i = exp(m_i - M)/S and gets the full
    attn_applied vector and the full attn_weights output.
  - comb_W is sharded over output rows (128/core): each core computes
    its slice of x = relu(comb_W @ [embedded; attn_applied] + b).
  - W_ih / W_hh are sharded over input columns matching the x / h0
    slice each core owns, producing partial LSTM gates (4096).
  - AllGather #2 carries the gate partials (16KB/rank); every core
    sums them, runs the LSTM elementwise math and the tiny output
    layer + log_softmax redundantly.  Host takes core 0's outputs.

Every weight byte is read from HBM exactly once across the chip
(~11.4 MB/core), which is the memory roofline for this problem.
All weights are pre-packed on the host into SBUF-ready [128, N]
layouts so every big DMA is a plain contiguous 128-partition copy.
"""

import numpy as np

NCORES = 8
V = 29
H = 1024
L = 4096
LSH = L // NCORES    # 512 logits per core
HSH = H // NCORES    # 128 hidden per core
CTR = 1544           # contrib floats: 1024 u + 1 s + 1 m + 512 logits + 6 pad

_CACHE = {}


def _pack_k(mat, ncols):
    """[K, ncols] (K = 128*nk) -> [128, nk*ncols]; block kt lands at
    columns [kt*ncols, (kt+1)*ncols) so sbuf[:, kt*ncols+c] = mat[kt*128+p, c]."""
    K = mat.shape[0]
    nk = K // 128
    return np.ascontiguousarray(
        mat.reshape(nk, 128, ncols).transpose(1, 0, 2).reshape(128, nk * ncols)
    )


def _build_nc():
    import concourse.bacc as bacc
    import concourse.mybir as mybir
    from concourse import tile

    F32 = mybir.dt.float32
    I32 = mybir.dt.int32
    AF = mybir.ActivationFunctionType
    ALU = mybir.AluOpType
    AX = mybir.AxisListType

    nc = bacc.Bacc(
        "TRN2", target_bir_lowering=False, debug=False, num_devices=NCORES
    )

    def inp(name, shape, dt=F32):
        return nc.dram_tensor(name, shape, dt, kind="ExternalInput")

    tok_d = inp("tok", [1, 1], I32)
    emb_d = inp("emb", [V, H])
    h0col_d = inp("h0col", [128, 8])
    c0col_d = inp("c0col", [128, 8])
    h0sl_d = inp("h0sl", [128, 1])
    ab_d = inp("ab", [128, 4])
    cb_d = inp("cb", [128, 1])
    bg_d = inp("bg", [128, 32])
    ob_d = inp("ob", [1, V])
    aW_d = [inp(f"aW{j}", [128, 2048]) for j in range(4)]
    enc_d = [inp(f"enc{j}", [128, 2048]) for j in range(2)]
    cW_d = inp("cW", [128, 2048])
    wih_d = [inp(f"wih{j}", [128, 2048]) for j in range(2)]
    whh_d = [inp(f"whh{j}", [128, 2048]) for j in range(2)]
    oW_d = inp("oW", [128, 8 * V])

    o_lp = nc.dram_tensor("logp", [1, V], F32, kind="ExternalOutput")
    o_h = nc.dram_tensor("hout", [128, 8], F32, kind="ExternalOutput")
    o_c = nc.dram_tensor("cout", [128, 8], F32, kind="ExternalOutput")
    o_aw = nc.dram_tensor("awout", [NCORES, LSH], F32, kind="ExternalOutput")

    iota_d = nc.inline_tensor(
        np.arange(V, dtype=np.float32).reshape(V, 1), name="iota29"
    )
    ident_d = nc.inline_tensor(np.eye(128, dtype=np.float32), name="ident128")

    RG = [list(range(NCORES))]

    with tile.TileContext(nc) as tc:
        with (
            tc.tile_pool(name="wts", bufs=1) as wp,
            tc.tile_pool(name="sml", bufs=1) as sp,
            tc.tile_pool(name="psum", bufs=5, space="PSUM") as pp,
            tc.tile_pool(name="psums", bufs=1, space="PSUM") as pps,
            tc.tile_pool(name="dram", bufs=1, space="DRAM") as dp,
        ):
            # ---------- small input loads (scalar HWDGE ring) ----------
            emb_sb = wp.tile([V, H], F32, tag="emb")
            nc.scalar.dma_start(emb_sb[:], emb_d.ap())
            h0c = wp.tile([128, 8], F32, tag="h0c")
            nc.scalar.dma_start(h0c[:], h0col_d.ap())
            c0c = wp.tile([128, 8], F32, tag="c0c")
            nc.scalar.dma_start(c0c[:], c0col_d.ap())
            h0s = wp.tile([128, 1], F32, tag="h0s")
            nc.scalar.dma_start(h0s[:], h0sl_d.ap())
            ab = wp.tile([128, 4], F32, tag="ab")
            nc.scalar.dma_start(ab[:], ab_d.ap())
            cb = wp.tile([128, 1], F32, tag="cb")
            nc.scalar.dma_start(cb[:], cb_d.ap())
            bg = wp.tile([128, 32], F32, tag="bg")
            nc.scalar.dma_start(bg[:], bg_d.ap())
            ob = wp.tile([1, V], F32, tag="ob")
            nc.scalar.dma_start(ob[:], ob_d.ap())
            iota = wp.tile([V, 1], F32, tag="iota")
            nc.scalar.dma_start(iota[:], iota_d.ap())
            tok_i = sp.tile([1, 1], I32, tag="tok_i")
            nc.scalar.dma_start(tok_i[:], tok_d.ap())
            oW = wp.tile([128, 8 * V], F32, tag="oW")
            nc.scalar.dma_start(oW[:], oW_d.ap())
            ident = wp.tile([128, 128], F32, tag="ident")
            nc.scalar.dma_start(ident[:], ident_d.ap())

            # ---------- big weight streams (sync HWDGE ring, in
            # critical-path order) ----------
            aW = [wp.tile([128, 2048], F32, tag=f"aW{j}", name=f"aW{j}_sb") for j in range(4)]
            encs = [wp.tile([128, 2048], F32, tag=f"enc{j}", name=f"enc{j}_sb") for j in range(2)]
            cW = wp.tile([128, 2048], F32, tag="cW")
            whh = [wp.tile([128, 2048], F32, tag=f"whh{j}", name=f"whh{j}_sb") for j in range(2)]
            wih = [wp.tile([128, 2048], F32, tag=f"wih{j}", name=f"wih{j}_sb") for j in range(2)]
            for j in range(4):
                nc.sync.dma_start(aW[j][:], aW_d[j].ap())
            for j in range(2):
                nc.sync.dma_start(encs[j][:], enc_d[j].ap())
            nc.sync.dma_start(cW[:], cW_d.ap())
            for j in range(2):
                nc.sync.dma_start(whh[j][:], whh_d[j].ap())
            for j in range(2):
                nc.sync.dma_start(wih[j][:], wih_d[j].ap())

            # ---------- constants ----------
            ones29 = sp.tile([1, V], F32, tag="ones29")
            nc.vector.memset(ones29[:], 1.0)
            onesP = sp.tile([1, 128], F32, tag="onesP")
            nc.vector.memset(onesP[:], 1.0)
            onesPc = sp.tile([128, 1], F32, tag="onesPc")
            nc.vector.memset(onesPc[:], 1.0)
            ones8r = sp.tile([1, 8], F32, tag="ones8r")
            nc.vector.memset(ones8r[:], 1.0)
            ones8c = sp.tile([8, 1], F32, tag="ones8c")
            nc.vector.memset(ones8c[:], 1.0)
            zpad = sp.tile([1, 6], F32, tag="zpad")
            nc.vector.memset(zpad[:], 0.0)

            # ---------- embedded = emb[tok] via on-device one-hot ----------
            tok_f = sp.tile([1, 1], F32, tag="tok_f")
            nc.vector.tensor_copy(tok_f[:], tok_i[:])
            ps_tb = pps.tile([V, 1], F32, tag="pss")
            nc.tensor.matmul(ps_tb[:], ones29[:], tok_f[:], start=True, stop=True)
            tokb = sp.tile([V, 1], F32, tag="tokb")
            nc.vector.tensor_copy(tokb[:], ps_tb[:])
            onehot = sp.tile([V, 1], F32, tag="onehot")
            nc.vector.tensor_tensor(onehot[:], iota[:], tokb[:], ALU.is_equal)
            ps_emb = pp.tile([128, 8], F32, tag="ps")
            for ht in range(8):
                nc.tensor.matmul(
                    ps_emb[:, ht : ht + 1],
                    emb_sb[:, ht * 128 : (ht + 1) * 128],
                    onehot[:],
                    start=True,
                    stop=True,
                )
            cat2 = sp.tile([128, 16], F32, tag="cat2")
            nc.vector.tensor_copy(cat2[:, 0:8], ps_emb[:])

            # ---------- attention logits (this core's 512 rows) ----------
            # one accumulation group per PSUM bank at a time -> mt outer
            ps_lg = pp.tile([128, 4], F32, tag="ps")
            for mt in range(4):
                for kt in range(16):
                    rhs = cat2[:, kt : kt + 1] if kt < 8 else h0c[:, kt - 8 : kt - 7]
                    a = aW[kt // 4]
                    off = (kt % 4) * 512
                    nc.tensor.matmul(
                        ps_lg[:, mt : mt + 1],
                        a[:, off + mt * 128 : off + (mt + 1) * 128],
                        rhs,
                        start=(kt == 0),
                        stop=(kt == 15),
                    )
            lgc = sp.tile([128, 4], F32, tag="lgc")
            nc.vector.tensor_add(lgc[:], ps_lg[:], ab[:])

            # local max over the 512 logits: free-dim reduce, PE transpose,
            # then free-dim reduce again (cross-partition max without gpsimd)
            rmax = sp.tile([128, 1], F32, tag="rmax")
            nc.vector.tensor_reduce(rmax[:], lgc[:], AX.X, ALU.max)
            ps_t = pps.tile([1, 128], F32, tag="psst")
            nc.tensor.transpose(ps_t[:], rmax[:], ident[:])
            rmrow = sp.tile([1, 128], F32, tag="rmrow")
            nc.vector.tensor_copy(rmrow[:], ps_t[:])
            negm = sp.tile([1, 1], F32, tag="negm")
            nc.vector.tensor_reduce(negm[:], rmrow[:], AX.X, ALU.max, negate=True)
            ps_nb = pps.tile([128, 1], F32, tag="pss")
            nc.tensor.matmul(ps_nb[:], onesP[:], negm[:], start=True, stop=True)
            negmb = sp.tile([128, 1], F32, tag="negmb")
            nc.vector.tensor_copy(negmb[:], ps_nb[:])
            expw = sp.tile([128, 4], F32, tag="expw")
            rowsum = sp.tile([128, 1], F32, tag="rowsum")
            nc.scalar.activation(
                expw[:], lgc[:], AF.Exp, bias=negmb[:], accum_out=rowsum[:]
            )
            ps_s = pps.tile([1, 1], F32, tag="pss")
            nc.tensor.matmul(ps_s[:], rowsum[:], onesPc[:], start=True, stop=True)
            sm = sp.tile([1, 2], F32, tag="sm")
            nc.vector.tensor_copy(sm[:, 0:1], ps_s[:])
            nc.vector.tensor_scalar_mul(sm[:, 1:2], negm[:], -1.0)

            # ---------- u = exp(logits - m) @ enc_shard ----------
            ps_u = pp.tile([128, 8], F32, tag="ps")
            for ht in range(8):
                for kt in range(4):
                    e = encs[kt // 2]
                    off = (kt % 2) * 1024
                    nc.tensor.matmul(
                        ps_u[:, ht : ht + 1],
                        e[:, off + ht * 128 : off + (ht + 1) * 128],
                        expw[:, kt : kt + 1],
                        start=(kt == 0),
                        stop=(kt == 3),
                    )
            u_sb = sp.tile([128, 8], F32, tag="u_sb")
            nc.vector.tensor_copy(u_sb[:], ps_u[:])

            # ---------- AllGather #1: {u, s, m, logits} ----------
            contrib = dp.tile([CTR], F32, tag="contrib")
            gath1 = dp.tile([NCORES, CTR], F32, tag="gath1")
            nc.scalar.dma_start(
                contrib[0:1024].rearrange("(f p) -> p f", p=128), u_sb[:]
            )
            nc.scalar.dma_start(
                contrib[1024:1026].rearrange("(a b) -> a b", a=1), sm[:]
            )
            nc.scalar.dma_start(
                contrib[1026:1538].rearrange("(f p) -> p f", p=128), lgc[:]
            )
            nc.scalar.dma_start(
                contrib[1538:1544].rearrange("(a b) -> a b", a=1), zpad[:]
            )
            nc.gpsimd.collective_compute(
                "AllGather",
                ALU.bypass,
                replica_groups=RG,
                ins=[contrib[:].opt()],
                outs=[gath1[:].opt()],
            )
            g1 = sp.tile([NCORES, CTR], F32, tag="g1")
            nc.scalar.dma_start(g1[:], gath1[:])

            # ---------- softmax rescale coefficients c_i = exp(m_i-M)/S ----
            m_col = g1[0:8, 1025:1026]
            s_col = g1[0:8, 1024:1025]
            ps_t8 = pps.tile([1, 8], F32, tag="psst")
            nc.tensor.transpose(ps_t8[:], m_col, ident[0:8, 0:8])
            mrow = sp.tile([1, 8], F32, tag="mrow")
            nc.vector.tensor_copy(mrow[:], ps_t8[:])
            negM = sp.tile([1, 1], F32, tag="negM")
            nc.vector.tensor_reduce(negM[:], mrow[:], AX.X, ALU.max, negate=True)
            ps_nM = pps.tile([8, 1], F32, tag="pss8")
            nc.tensor.matmul(ps_nM[:], ones8r[:], negM[:], start=True, stop=True)
            negMb = sp.tile([8, 1], F32, tag="negMb")
            nc.vector.tensor_copy(negMb[:], ps_nM[:])
            d_col = sp.tile([8, 1], F32, tag="d_col")
            nc.scalar.activation(d_col[:], m_col, AF.Exp, bias=negMb[:])
            sd_col = sp.tile([8, 1], F32, tag="sd_col")
            nc.vector.tensor_mul(sd_col[:], d_col[:], s_col)
            ps_S = pps.tile([1, 1], F32, tag="pss")
            nc.tensor.matmul(ps_S[:], sd_col[:], ones8c[:], start=True, stop=True)
            S_sb = sp.tile([1, 1], F32, tag="S_sb")
            nc.vector.tensor_copy(S_sb[:], ps_S[:])
            rS = sp.tile([1, 1], F32, tag="rS")
            nc.vector.reciprocal(rS[:], S_sb[:])
            ps_rS = pps.tile([8, 1], F32, tag="pss8")
            nc.tensor.matmul(ps_rS[:], ones8r[:], rS[:], start=True, stop=True)
            rSb = sp.tile([8, 1], F32, tag="rSb")
            nc.vector.tensor_copy(rSb[:], ps_rS[:])
            c_col = sp.tile([8, 1], F32, tag="c_col")
            nc.vector.tensor_mul(c_col[:], d_col[:], rSb[:])

            # ---------- attn_applied = sum_i c_i * u_i ----------
            ps_aa = pp.tile([128, 8], F32, tag="ps")
            for ht in range(8):
                nc.tensor.matmul(
                    ps_aa[:, ht : ht + 1],
                    g1[0:8, ht * 128 : (ht + 1) * 128],
                    c_col[:],
                    start=True,
                    stop=True,
                )
            nc.vector.tensor_copy(cat2[:, 8:16], ps_aa[:])

            # ---------- attn_weights output (full, exact softmax) ----------
            negm8 = sp.tile([8, 1], F32, tag="negm8")
            nc.vector.tensor_scalar_mul(negm8[:], m_col, -1.0)
            aw = sp.tile([8, LSH], F32, tag="aw")
            nc.scalar.activation(aw[:], g1[0:8, 1026:1538], AF.Exp, bias=negm8[:])
            nc.vector.tensor_scalar_mul(aw[:], aw[:], c_col[:])
            nc.scalar.dma_start(o_aw.ap(), aw[:])

            # ---------- x slice = relu(comb_W_rows @ cat2 + b) ----------
            ps_x = pp.tile([128, 1], F32, tag="ps")
            for kt in range(16):
                nc.tensor.matmul(
                    ps_x[:],
                    cW[:, kt * 128 : (kt + 1) * 128],
                    cat2[:, kt : kt + 1],
                    start=(kt == 0),
                    stop=(kt == 15),
                )
            x_col = sp.tile([128, 1], F32, tag="x_col")
            nc.scalar.activation(x_col[:], ps_x[:], AF.Relu, bias=cb[:])

            # ---------- partial LSTM gates ----------
            # W_hh @ h0_slice runs early (h0 is an input); W_ih @ x_slice
            # waits for x.  Separate PSUM tiles keep the accumulation
            # groups bank-sequential; summed on the way to SBUF.
            ps_gh = pp.tile([128, 32], F32, tag="ps")
            for j in range(2):
                for mt in range(16):
                    col = j * 16 + mt
                    nc.tensor.matmul(
                        ps_gh[:, col : col + 1],
                        whh[j][:, mt * 128 : (mt + 1) * 128],
                        h0s[:],
                        start=True,
                        stop=True,
                    )
            ghh_sb = sp.tile([128, 32], F32, tag="ghh_sb")
            nc.vector.tensor_copy(ghh_sb[:], ps_gh[:])
            ps_g = pp.tile([128, 32], F32, tag="ps")
            for j in range(2):
                for mt in range(16):
                    col = j * 16 + mt
                    nc.tensor.matmul(
                        ps_g[:, col : col + 1],
                        wih[j][:, mt * 128 : (mt + 1) * 128],
                        x_col[:],
                        start=True,
                        stop=True,
                    )
            gp_sb = sp.tile([128, 32], F32, tag="gp_sb")
            nc.vector.tensor_add(gp_sb[:], ps_g[:], ghh_sb[:])

            # ---------- AllGather #2: gate partials ----------
            contrib2 = dp.tile([4096], F32, tag="contrib2")
            gath2 = dp.tile([NCORES, 4096], F32, tag="gath2")
            nc.scalar.dma_start(
                contrib2[:].rearrange("(f p) -> p f", p=128), gp_sb[:]
            )
            nc.gpsimd.collective_compute(
                "AllGather",
                ALU.bypass,
                replica_groups=RG,
                ins=[contrib2[:].opt()],
                outs=[gath2[:].opt()],
            )
            g2 = sp.tile([128, 256], F32, tag="g2")
            nc.scalar.dma_start(
                g2[:].rearrange("p (r c) -> p r c", r=8),
                gath2[:].rearrange("r (c p) -> p r c", p=128),
            )

            # ---------- sum gate partials + bias ----------
            gates = sp.tile([128, 32], F32, tag="gates")
            nc.vector.tensor_add(gates[:], g2[:, 0:32], g2[:, 32:64])
            for r in range(2, 8):
                nc.vector.tensor_add(gates[:], gates[:], g2[:, r * 32 : (r + 1) * 32])
            nc.vector.tensor_add(gates[:], gates[:], bg[:])

            # ---------- LSTM elementwise (gate order i, f, g, o) ----------
            sig = sp.tile([128, 24], F32, tag="sig")
            nc.scalar.activation(sig[:, 0:8], gates[:, 0:8], AF.Sigmoid)
            nc.scalar.activation(sig[:, 8:16], gates[:, 8:16], AF.Sigmoid)
            nc.scalar.activation(sig[:, 16:24], gates[:, 24:32], AF.Sigmoid)
            gt = sp.tile([128, 8], F32, tag="gt")
            nc.scalar.activation(gt[:], gates[:, 16:24], AF.Tanh)
            cnew = sp.tile([128, 8], F32, tag="cnew")
            t1 = sp.tile([128, 8], F32, tag="t1")
            nc.vector.tensor_mul(cnew[:], sig[:, 8:16], c0c[:])
            nc.vector.tensor_mul(t1[:], sig[:, 0:8], gt[:])
            nc.vector.tensor_add(cnew[:], cnew[:], t1[:])
            tcn = sp.tile([128, 8], F32, tag="tcn")
            nc.scalar.activation(tcn[:], cnew[:], AF.Tanh)
            hnew = sp.tile([128, 8], F32, tag="hnew")
            nc.vector.tensor_mul(hnew[:], sig[:, 16:24], tcn[:])
            nc.scalar.dma_start(o_h.ap(), hnew[:])
            nc.scalar.dma_start(o_c.ap(), cnew[:])

            # ---------- output layer + log_softmax ----------
            ps_o = pps.tile([1, V], F32, tag="pss")
            for kt in range(8):
                nc.tensor.matmul(
                    ps_o[:],
                    hnew[:, kt : kt + 1],
                    oW[:, kt * V : (kt + 1) * V],
                    start=(kt == 0),
                    stop=(kt == 7),
                )
            orow = sp.tile([1, V], F32, tag="orow")
            nc.vector.tensor_add(orow[:], ps_o[:], ob[:])
            negmx = sp.tile([1, 1], F32, tag="negmx")
            nc.vector.tensor_reduce(negmx[:], orow[:], AX.X, ALU.max, negate=True)
            erow = sp.tile([1, V], F32, tag="erow")
            sume = sp.tile([1, 1], F32, tag="sume")
            nc.scalar.activation(
                erow[:], orow[:], AF.Exp, bias=negmx[:], accum_out=sume[:]
            )
            lse = sp.tile([1, 1], F32, tag="lse")
            nc.scalar.activation(lse[:], sume[:], AF.Ln)
            lp = sp.tile([1, V], F32, tag="lp")
            nc.vector.tensor_scalar(
                lp[:], orow[:], negmx[:], lse[:], ALU.add, ALU.subtract
            )
            nc.scalar.dma_start(o_lp.ap(), lp[:])

    nc.finalize()  # Bacc.finalize = compile passes (reg alloc etc) + freeze
    return nc


def prep_in_maps(inputs):
    tok = np.asarray(inputs["input_tok"]).reshape(1, 1).astype(np.int32)
    h0 = np.asarray(inputs["hidden_state"], dtype=np.float32).reshape(H)
    c0 = np.asarray(inputs["cell_state"], dtype=np.float32).reshape(H)
    enc = np.asarray(inputs["encoder_outputs"], dtype=np.float32)
    emb = np.ascontiguousarray(np.asarray(inputs["emb"], dtype=np.float32))
    aW = np.asarray(inputs["attn_W"], dtype=np.float32)
    abv = np.asarray(inputs["attn_b"], dtype=np.float32)
    cWm = np.asarray(inputs["comb_W"], dtype=np.float32)
    cbv = np.asarray(inputs["comb_b"], dtype=np.float32)
    wih = np.asarray(inputs["W_ih"], dtype=np.float32)
    whh = np.asarray(inputs["W_hh"], dtype=np.float32)
    bgv = np.asarray(inputs["b_ih"], dtype=np.float32) + np.asarray(
        inputs["b_hh"], dtype=np.float32
    )
    oWm = np.asarray(inputs["out_W"], dtype=np.float32)
    obv = np.asarray(inputs["out_b"], dtype=np.float32)

    h0col = np.ascontiguousarray(h0.reshape(8, 128).T)
    c0col = np.ascontiguousarray(c0.reshape(8, 128).T)
    bgcol = np.ascontiguousarray(bgv.reshape(32, 128).T)
    oWp = _pack_k(np.ascontiguousarray(oWm.T), V)           # [128, 8*29]
    obr = np.ascontiguousarray(obv.reshape(1, V))

    in_maps = []
    for i in range(NCORES):
        aWp = _pack_k(np.ascontiguousarray(aW[i * LSH : (i + 1) * LSH, :].T), LSH)
        encp = _pack_k(enc[i * LSH : (i + 1) * LSH, :], H)  # [128, 4096]
        cWp = _pack_k(
            np.ascontiguousarray(cWm[i * HSH : (i + 1) * HSH, :].T), HSH
        )                                                    # [128, 2048]
        wihp = np.ascontiguousarray(wih[:, i * HSH : (i + 1) * HSH].T)  # [128,4096]
        whhp = np.ascontiguousarray(whh[:, i * HSH : (i + 1) * HSH].T)
        m = {
            "tok": tok,
            "emb": emb,
            "h0col": h0col,
            "c0col": c0col,
            "h0sl": np.ascontiguousarray(
                h0[i * HSH : (i + 1) * HSH].reshape(128, 1)
            ),
            "ab": np.ascontiguousarray(
                abv[i * LSH : (i + 1) * LSH].reshape(4, 128).T
            ),
            "cb": np.ascontiguousarray(
                cbv[i * HSH : (i + 1) * HSH].reshape(128, 1)
            ),
            "bg": bgcol,
            "ob": obr,
            "cW": cWp,
            "oW": oWp,
        }
        for j in range(4):
            m[f"aW{j}"] = np.ascontiguousarray(aWp[:, j * 2048 : (j + 1) * 2048])
        for j in range(2):
            m[f"enc{j}"] = np.ascontiguousarray(encp[:, j * 2048 : (j + 1) * 2048])
            m[f"wih{j}"] = np.ascontiguousarray(wihp[:, j * 2048 : (j + 1) * 2048])
            m[f"whh{j}"] = np.ascontiguousarray(whhp[:, j * 2048 : (j + 1) * 2048])
        in_maps.append(m)
    return in_maps


def unpack_outputs(res0):
    log_probs = np.asarray(res0["logp"], dtype=np.float32).reshape(1, V)
    h_new = np.ascontiguousarray(
        np.asarray(res0["hout"], dtype=np.float32).T
    ).reshape(1, 1, H)
    c_new = np.ascontiguousarray(
        np.asarray(res0["cout"], dtype=np.float32).T
    ).reshape(1, 1, H)
    attn_w = np.asarray(res0["awout"], dtype=np.float32).reshape(1, L)
    return (log_probs, h_new, c_new, attn_w)


def get_nc():
    if "nc" not in _CACHE:
        _CACHE["nc"] = _build_nc()
    return _CACHE["nc"]


def kernel(**inputs):
    from concourse import bass_utils

    nc = get_nc()
    in_maps = prep_in_maps(inputs)
    out = bass_utils.run_bass_kernel_spmd(nc, in_maps, core_ids=list(range(NCORES)))
    return unpack_outputs(out.results[0])


if __name__ == "__main__":
    nc = _build_nc()
    print("built ok; instructions:", sum(len(bb.instructions) for bb in nc.main_func.blocks))


# revision 17
# speedup vs baseline: 1.6953x; 1.6953x over previous
"""AttentionDecoderRNN single-step decode on 8 TRN2 NeuronCores.

Strategy (tensor-parallel matvec, 1 AllGather + 1 AllReduce):
  - attn_W sharded over output rows (512 logits/core); encoder_outputs
    sharded over the same rows, so each core computes an *unnormalized*
    attention-value partial u_i = sum_j exp(logit_ij - m_i) * enc_i[j,:]
    plus local stats (m_i, s_i) -- flash-attention style.
  - AllGather #1 carries {u_i, s_i, m_i, logits_i}; every core then
    reconstructs the exact global softmax via c_i = exp(m_i - M)/S and
    gets attn_applied plus the full attn_weights output.
  - comb_W sharded over output rows (128/core) -> local x slice (relu).
  - W_ih / W_hh sharded over input columns matching the x / h0 slice
    each core owns -> partial LSTM gates (4096).
  - AllReduce #2 (add) sums the gate partials; every core runs the LSTM
    elementwise math + output layer + log_softmax redundantly.

Performance notes (from trace analysis of the f32 v1):
  - fp32 matmuls lower to 2x(LDWEIGHTS+MATMUL) at ~430ns per 128x128
    tile -> PE consumes weights at only ~149GB/s.  v2 stores all big
    weights in bf16 (halves DMA bytes; rel err ~2.7e-3, gate is 2e-2)
    and flips the two big matvecs so the weight matrix is the *moving*
    operand: one matmul covers [128K x 512N] (16 instructions for the
    whole attention logit matvec instead of 64).
  - A single HWDGE ring sustains only ~146GB/s, so the weight streams
    are spread across the vector / tensor / gpsimd rings.  The sync
    ring is kept free for latency-critical mid-kernel DMAs (collective
    bounce traffic); the scalar ring is avoided entirely (it carries
    ~39k 4-byte event-semaphore packets and showed 15-30us completion
    latencies).
  - Gate partials go through AllReduce (CCE adds) instead of AllGather
    + on-chip sum: the strided 8-rank interleaved readback cost ~36us.
  - Activation tables (Exp/Sigmoid/Tanh/Ln) are warmed with dummy ops
    at t=0 so the 1.3us ACT_TABLE_LOADs stay off the critical path.
"""

import numpy as np

NCORES = 8
V = 29
H = 1024
L = 4096
LSH = L // NCORES    # 512 logits per core
HSH = H // NCORES    # 128 hidden per core
CTR = 1544           # contrib floats: 1024 u + 1 s + 1 m + 512 logits + 6 pad

_CACHE = {}


def _bf16(x):
    import ml_dtypes

    return np.ascontiguousarray(
        np.asarray(x, dtype=np.float32).astype(ml_dtypes.bfloat16)
    )


def _pack_k(mat, ncols):
    """[K, ncols] (K = 128*nk) -> [128, nk*ncols]; block kt lands at
    columns [kt*ncols, (kt+1)*ncols) so out[p, kt*ncols+c] = mat[kt*128+p, c]."""
    K = mat.shape[0]
    nk = K // 128
    return np.ascontiguousarray(
        mat.reshape(nk, 128, ncols).transpose(1, 0, 2).reshape(128, nk * ncols)
    )


def _build_nc():
    import concourse.bacc as bacc
    import concourse.mybir as mybir
    from concourse import tile

    F32 = mybir.dt.float32
    BF16 = mybir.dt.bfloat16
    I32 = mybir.dt.int32
    AF = mybir.ActivationFunctionType
    ALU = mybir.AluOpType
    AX = mybir.AxisListType

    nc = bacc.Bacc(
        "TRN2", target_bir_lowering=False, debug=False, num_devices=NCORES
    )

    def inp(name, shape, dt=F32):
        return nc.dram_tensor(name, shape, dt, kind="ExternalInput")

    tok_d = inp("tok", [1, 1], I32)
    emb_d = inp("emb", [V, H], BF16)
    smf_d = inp("smf", [128, 41])            # c0col | cb | bg   (f32)
    rowf_d = inp("rowf", [1, 541])           # ob pad'd row      (f32)
    smb_d = inp("smb", [128, 9], BF16)       # h0col | h0slice   (bf16)
    abb_d = inp("abb", [1, LSH], BF16)       # attn_b slice row  (bf16)
    aW_d = [inp(f"aW{j}", [128, 2048], BF16) for j in range(4)]
    enc_d = [inp(f"enc{j}", [128, 2048], BF16) for j in range(2)]
    cW_d = inp("cW", [128, 2048], BF16)
    wih_d = [inp(f"wih{j}", [128, 2048], BF16) for j in range(2)]
    whh_d = [inp(f"whh{j}", [128, 2048], BF16) for j in range(2)]
    oW_d = inp("oW", [128, 8 * V], BF16)

    o_lp = nc.dram_tensor("logp", [1, V], F32, kind="ExternalOutput")
    o_h = nc.dram_tensor("hout", [128, 8], F32, kind="ExternalOutput")
    o_c = nc.dram_tensor("cout", [128, 8], F32, kind="ExternalOutput")
    o_aw = nc.dram_tensor("awout", [NCORES, LSH], F32, kind="ExternalOutput")

    iota_d = nc.inline_tensor(
        np.arange(V, dtype=np.float32).reshape(V, 1), name="iota29"
    )
    id8_d = nc.inline_tensor(np.eye(8, dtype=np.float32), name="ident8")


    RG = [list(range(NCORES))]

    with tile.TileContext(nc) as tc:
        with (
            tc.tile_pool(name="wts", bufs=1) as wp,
            tc.tile_pool(name="sml", bufs=1) as sp,
            tc.tile_pool(name="psum", bufs=4, space="PSUM") as pp,
            tc.tile_pool(name="psumu", bufs=1, space="PSUM") as ppu,
            tc.tile_pool(name="psums", bufs=1, space="PSUM") as pps,
            tc.tile_pool(name="dram", bufs=1, space="DRAM") as dp,
        ):
            # ---------- small input loads (sync ring, all tiny) ----------
            emb_sb = wp.tile([V, H], BF16, tag="emb")
            nc.sync.dma_start(emb_sb[:], emb_d.ap())
            smf = wp.tile([128, 41], F32, tag="smf")
            nc.sync.dma_start(smf[:], smf_d.ap())
            rowf = wp.tile([1, 541], F32, tag="rowf")
            nc.sync.dma_start(rowf[:], rowf_d.ap())
            smb = wp.tile([128, 9], BF16, tag="smb")
            nc.sync.dma_start(smb[:], smb_d.ap())
            abb = wp.tile([1, LSH], BF16, tag="abb")
            nc.sync.dma_start(abb[:], abb_d.ap())
            iota = wp.tile([V, 1], F32, tag="iota")
            nc.sync.dma_start(iota[:], iota_d.ap())
            id8 = wp.tile([8, 8], F32, tag="id8")
            nc.sync.dma_start(id8[:], id8_d.ap())
            tok_i = sp.tile([1, 1], I32, tag="tok_i")
            nc.sync.dma_start(tok_i[:], tok_d.ap())
            oW = wp.tile([128, 8 * V], BF16, tag="oW")
            nc.sync.dma_start(oW[:], oW_d.ap())

            c0c = smf[:, 0:8]
            cb = smf[:, 8:9]
            bg = smf[:, 9:41]
            ob = rowf[:, 512:541]
            h0c = smb[:, 0:8]
            h0s = smb[:, 8:9]

            # ---------- big weight streams, spread across rings ----------
            aW = [wp.tile([128, 2048], BF16, tag=f"aW{j}", name=f"aW{j}_sb") for j in range(4)]
            encs = [wp.tile([128, 2048], BF16, tag=f"enc{j}", name=f"enc{j}_sb") for j in range(2)]
            cW = wp.tile([128, 2048], BF16, tag="cW")
            whh = [wp.tile([128, 2048], BF16, tag=f"whh{j}", name=f"whh{j}_sb") for j in range(2)]
            wih = [wp.tile([128, 2048], BF16, tag=f"wih{j}", name=f"wih{j}_sb") for j in range(2)]
            # sync HWDGE ring: attn weights (head of the dependency chain)
            # then comb weights; gpsimd SWDGE ring: enc + LSTM weights.
            for j in range(4):
                nc.sync.dma_start(aW[j][:], aW_d[j].ap())
            nc.sync.dma_start(cW[:], cW_d.ap())
            for j in range(2):
                nc.gpsimd.dma_start(encs[j][:], enc_d[j].ap())
            for j in range(2):
                nc.gpsimd.dma_start(whh[j][:], whh_d[j].ap())
            for j in range(2):
                nc.gpsimd.dma_start(wih[j][:], wih_d[j].ap())

            # ---------- constants + activation-table warmup ----------
            ones29 = sp.tile([1, V], F32, tag="ones29")
            nc.vector.memset(ones29[:], 1.0)
            ones8r = sp.tile([1, 8], F32, tag="ones8r")
            nc.vector.memset(ones8r[:], 1.0)
            ones8c = sp.tile([8, 1], F32, tag="ones8c")
            nc.vector.memset(ones8c[:], 1.0)
            one_bf = sp.tile([1, 1], BF16, tag="one_bf")
            nc.vector.memset(one_bf[:], 1.0)
            zpad = sp.tile([1, 6], F32, tag="zpad")
            nc.vector.memset(zpad[:], 0.0)
            warm = sp.tile([1, 4], F32, tag="warm")
            nc.vector.memset(warm[:], 0.5)
            for fn in (AF.Exp, AF.Sigmoid, AF.Tanh, AF.Ln):
                nc.scalar.activation(warm[:], warm[:], fn)

            # ---------- embedded = emb[tok] via on-device one-hot ----------
            tok_f = sp.tile([1, 1], F32, tag="tok_f")
            nc.vector.tensor_copy(tok_f[:], tok_i[:])
            ps_tb = pps.tile([V, 1], F32, tag="pss")
            nc.tensor.matmul(ps_tb[:], ones29[:], tok_f[:], start=True, stop=True)
            tokb = sp.tile([V, 1], F32, tag="tokb")
            nc.vector.tensor_copy(tokb[:], ps_tb[:])
            onehot = sp.tile([V, 1], F32, tag="onehot")
            nc.vector.tensor_tensor(onehot[:], iota[:], tokb[:], ALU.is_equal)
            ohb = sp.tile([V, 1], BF16, tag="ohb")
            nc.vector.tensor_copy(ohb[:], onehot[:])
            ps_emb = pp.tile([128, 8], F32, tag="ps")
            for ht in range(8):
                nc.tensor.matmul(
                    ps_emb[:, ht : ht + 1],
                    emb_sb[:, ht * 128 : (ht + 1) * 128],
                    ohb[:],
                    start=True,
                    stop=True,
                )
            cat2 = sp.tile([128, 16], BF16, tag="cat2")
            nc.vector.tensor_copy(cat2[:, 0:8], ps_emb[:])

            # ---------- attention logits (row form, weights moving) ------
            # logits[l] = sum_d cat1[d] * attn_W[l, d] + attn_b[l]
            ps_lg = ppu.tile([1, LSH], F32, tag="pslg")
            for kt in range(16):
                vec = cat2[:, kt : kt + 1] if kt < 8 else h0c[:, kt - 8 : kt - 7]
                nc.tensor.matmul(
                    ps_lg[:],
                    vec,
                    aW[kt // 4][:, (kt % 4) * 512 : (kt % 4 + 1) * 512],
                    start=(kt == 0),
                    stop=False,
                )
            nc.tensor.matmul(ps_lg[:], one_bf[:], abb[:], start=False, stop=True)

            # local softmax stats on the row
            negm = sp.tile([1, 1], F32, tag="negm")
            nc.vector.tensor_reduce(negm[:], ps_lg[:], AX.X, ALU.max, negate=True)
            expr = sp.tile([1, LSH], F32, tag="expr")
            s_row = sp.tile([1, 1], F32, tag="s_row")
            nc.scalar.activation(
                expr[:], ps_lg[:], AF.Exp, bias=negm[:], accum_out=s_row[:]
            )
            sm = sp.tile([1, 2], F32, tag="sm")
            nc.vector.tensor_copy(sm[:, 0:1], s_row[:])
            nc.vector.tensor_scalar_mul(sm[:, 1:2], negm[:], -1.0)

            # transpose exp weights row -> column [128, 4] (f32: PSUM
            # writes must stay 4-byte aligned; cast to bf16 on the copy out)
            ps_tc = pp.tile([128, 4], F32, tag="ps")
            for j in range(4):
                nc.tensor.transpose(
                    ps_tc[:, j : j + 1], expr[:, j * 128 : (j + 1) * 128],
                    ones29[:, 0:1]
                )
            expc = sp.tile([128, 4], BF16, tag="expc")
            nc.vector.tensor_copy(expc[:], ps_tc[:])

            # ---------- u = expw @ enc_shard (row form) ----------
            ps_u = ppu.tile([1, H], F32, tag="psu")
            for nt in range(2):
                for kt in range(4):
                    nc.tensor.matmul(
                        ps_u[:, nt * 512 : (nt + 1) * 512],
                        expc[:, kt : kt + 1],
                        encs[kt // 2][:, (kt % 2) * 1024 + nt * 512 : (kt % 2) * 1024 + nt * 512 + 512],
                        start=(kt == 0),
                        stop=(kt == 3),
                    )

            # ---------- AllGather #1: {u, s, m, logits} ----------
            # (DMA cannot read PSUM -- stage the row results in SBUF,
            # splitting the single-partition copies across DVE and ACT)
            u_row = sp.tile([1, H], F32, tag="u_row")
            nc.vector.tensor_copy(u_row[:, 0:512], ps_u[:, 0:512])
            nc.scalar.copy(u_row[:, 512:1024], ps_u[:, 512:1024])
            lgr = sp.tile([1, LSH], F32, tag="lgr")
            nc.vector.tensor_copy(lgr[:], ps_lg[:])
            contrib = dp.tile([CTR], F32, tag="contrib")
            gath1 = dp.tile([NCORES, CTR], F32, tag="gath1")
            nc.sync.dma_start(
                contrib[0:1024].rearrange("(a b) -> a b", a=1), u_row[:]
            )
            nc.sync.dma_start(
                contrib[1024:1026].rearrange("(a b) -> a b", a=1), sm[:]
            )
            nc.sync.dma_start(
                contrib[1026:1538].rearrange("(a b) -> a b", a=1), lgr[:]
            )
            nc.sync.dma_start(
                contrib[1538:1544].rearrange("(a b) -> a b", a=1), zpad[:]
            )
            nc.gpsimd.collective_compute(
                "AllGather",
                ALU.bypass,
                replica_groups=RG,
                ins=[contrib[:].opt()],
                outs=[gath1[:].opt()],
            )
            g1 = sp.tile([NCORES, CTR], F32, tag="g1")
            nc.sync.dma_start(g1[:], gath1[:])

            # ---------- softmax rescale coefficients c_i = exp(m_i-M)/S --
            m_col = g1[0:8, 1025:1026]
            s_col = g1[0:8, 1024:1025]
            ps_t8 = pps.tile([1, 8], F32, tag="pss")
            nc.tensor.transpose(ps_t8[:], m_col, id8[:])
            mrow = sp.tile([1, 8], F32, tag="mrow")
            nc.vector.tensor_copy(mrow[:], ps_t8[:])
            negM = sp.tile([1, 1], F32, tag="negM")
            nc.vector.tensor_reduce(negM[:], mrow[:], AX.X, ALU.max, negate=True)
            ps_nM = pps.tile([8, 1], F32, tag="pss")
            nc.tensor.matmul(ps_nM[:], ones8r[:], negM[:], start=True, stop=True)
            negMb = sp.tile([8, 1], F32, tag="negMb")
            nc.vector.tensor_copy(negMb[:], ps_nM[:])
            d_col = sp.tile([8, 1], F32, tag="d_col")
            nc.scalar.activation(d_col[:], m_col, AF.Exp, bias=negMb[:])
            sd_col = sp.tile([8, 1], F32, tag="sd_col")
            nc.vector.tensor_mul(sd_col[:], d_col[:], s_col)
            ps_S = pps.tile([1, 1], F32, tag="pss")
            nc.tensor.matmul(ps_S[:], sd_col[:], ones8c[:], start=True, stop=True)
            S_sb = sp.tile([1, 1], F32, tag="S_sb")
            nc.vector.tensor_copy(S_sb[:], ps_S[:])
            rS = sp.tile([1, 1], F32, tag="rS")
            nc.vector.reciprocal(rS[:], S_sb[:])
            ps_rS = pps.tile([8, 1], F32, tag="pss")
            nc.tensor.matmul(ps_rS[:], ones8r[:], rS[:], start=True, stop=True)
            rSb = sp.tile([8, 1], F32, tag="rSb")
            nc.vector.tensor_copy(rSb[:], ps_rS[:])
            c_col = sp.tile([8, 1], F32, tag="c_col")
            nc.vector.tensor_mul(c_col[:], d_col[:], rSb[:])

            # ---------- attn_applied = sum_i c_i * u_i ----------
            ps_aa = pp.tile([128, 8], F32, tag="ps")
            for ht in range(8):
                nc.tensor.matmul(
                    ps_aa[:, ht : ht + 1],
                    g1[0:8, ht * 128 : (ht + 1) * 128],
                    c_col[:],
                    start=True,
                    stop=True,
                )
            nc.vector.tensor_copy(cat2[:, 8:16], ps_aa[:])

            # ---------- attn_weights output (full, exact softmax) --------
            negm8 = sp.tile([8, 1], F32, tag="negm8")
            nc.vector.tensor_scalar_mul(negm8[:], m_col, -1.0)
            aw = sp.tile([8, LSH], F32, tag="aw")
            nc.scalar.activation(aw[:], g1[0:8, 1026:1538], AF.Exp, bias=negm8[:])
            nc.vector.tensor_scalar_mul(aw[:], aw[:], c_col[:])
            nc.sync.dma_start(o_aw.ap(), aw[:])

            # ---------- x slice = relu(comb_W_rows @ cat2 + b) ----------
            ps_x = pp.tile([128, 1], F32, tag="ps")
            for kt in range(16):
                nc.tensor.matmul(
                    ps_x[:],
                    cW[:, kt * 128 : (kt + 1) * 128],
                    cat2[:, kt : kt + 1],
                    start=(kt == 0),
                    stop=(kt == 15),
                )
            x_col = sp.tile([128, 1], BF16, tag="x_col")
            nc.scalar.activation(x_col[:], ps_x[:], AF.Relu, bias=cb)

            # ---------- partial LSTM gates ----------
            # W_hh @ h0_slice runs early (h0 is an input); W_ih @ x_slice
            # waits for x.  Separate PSUM tiles keep the accumulation
            # groups bank-sequential; summed on the way to DRAM.
            ps_gh = pp.tile([128, 32], F32, tag="ps")
            for j in range(2):
                for mt in range(16):
                    col = j * 16 + mt
                    nc.tensor.matmul(
                        ps_gh[:, col : col + 1],
                        whh[j][:, mt * 128 : (mt + 1) * 128],
                        h0s,
                        start=True,
                        stop=True,
                    )
            ghh_sb = sp.tile([128, 32], F32, tag="ghh_sb")
            nc.vector.tensor_copy(ghh_sb[:], ps_gh[:])
            ps_g = pp.tile([128, 32], F32, tag="ps")
            for j in range(2):
                for mt in range(16):
                    col = j * 16 + mt
                    nc.tensor.matmul(
                        ps_g[:, col : col + 1],
                        wih[j][:, mt * 128 : (mt + 1) * 128],
                        x_col[:],
                        start=True,
                        stop=True,
                    )
            gp_sb = sp.tile([128, 32], F32, tag="gp_sb")
            nc.vector.tensor_add(gp_sb[:], ps_g[:], ghh_sb[:])

            # ---------- AllReduce #2: sum gate partials across cores -----
            contrib2 = dp.tile([4096], F32, tag="contrib2")
            gsum = dp.tile([4096], F32, tag="gsum")
            nc.sync.dma_start(
                contrib2[:].rearrange("(f p) -> p f", p=128), gp_sb[:]
            )
            nc.gpsimd.collective_compute(
                "AllReduce",
                ALU.add,
                replica_groups=RG,
                ins=[contrib2[:].opt()],
                outs=[gsum[:].opt()],
            )
            g2c = sp.tile([128, 32], F32, tag="g2c")
            nc.sync.dma_start(g2c[:], gsum[:].rearrange("(f p) -> p f", p=128))

            gates = sp.tile([128, 32], F32, tag="gates")
            nc.vector.tensor_add(gates[:], g2c[:], bg)

            # ---------- LSTM elementwise (gate order i, f, g, o) ----------
            sig = sp.tile([128, 24], F32, tag="sig")
            nc.scalar.activation(sig[:, 0:8], gates[:, 0:8], AF.Sigmoid)
            nc.scalar.activation(sig[:, 8:16], gates[:, 8:16], AF.Sigmoid)
            nc.scalar.activation(sig[:, 16:24], gates[:, 24:32], AF.Sigmoid)
            gt = sp.tile([128, 8], F32, tag="gt")
            nc.scalar.activation(gt[:], gates[:, 16:24], AF.Tanh)
            cnew = sp.tile([128, 8], F32, tag="cnew")
            t1 = sp.tile([128, 8], F32, tag="t1")
            nc.vector.tensor_mul(cnew[:], sig[:, 8:16], c0c)
            nc.vector.tensor_mul(t1[:], sig[:, 0:8], gt[:])
            nc.vector.tensor_add(cnew[:], cnew[:], t1[:])
            tcn = sp.tile([128, 8], F32, tag="tcn")
            nc.scalar.activation(tcn[:], cnew[:], AF.Tanh)
            hnew = sp.tile([128, 8], F32, tag="hnew")
            nc.vector.tensor_mul(hnew[:], sig[:, 16:24], tcn[:])
            nc.sync.dma_start(o_h.ap(), hnew[:])
            nc.sync.dma_start(o_c.ap(), cnew[:])

            # ---------- output layer + log_softmax ----------
            h_bf = sp.tile([128, 8], BF16, tag="h_bf")
            nc.vector.tensor_copy(h_bf[:], hnew[:])
            ps_o = pps.tile([1, V], F32, tag="pss")
            for kt in range(8):
                nc.tensor.matmul(
                    ps_o[:],
                    h_bf[:, kt : kt + 1],
                    oW[:, kt * V : (kt + 1) * V],
                    start=(kt == 0),
                    stop=(kt == 7),
                )
            orow = sp.tile([1, V], F32, tag="orow")
            nc.vector.tensor_add(orow[:], ps_o[:], ob)
            negmx = sp.tile([1, 1], F32, tag="negmx")
            nc.vector.tensor_reduce(negmx[:], orow[:], AX.X, ALU.max, negate=True)
            erow = sp.tile([1, V], F32, tag="erow")
            sume = sp.tile([1, 1], F32, tag="sume")
            nc.scalar.activation(
                erow[:], orow[:], AF.Exp, bias=negmx[:], accum_out=sume[:]
            )
            lse = sp.tile([1, 1], F32, tag="lse")
            nc.scalar.activation(lse[:], sume[:], AF.Ln)
            lp = sp.tile([1, V], F32, tag="lp")
            nc.vector.tensor_scalar(
                lp[:], orow[:], negmx[:], lse[:], ALU.add, ALU.subtract
            )
            nc.sync.dma_start(o_lp.ap(), lp[:])

    nc.finalize()  # Bacc.finalize = compile passes (reg alloc etc) + freeze
    return nc


def prep_in_maps(inputs):
    tok = np.asarray(inputs["input_tok"]).reshape(1, 1).astype(np.int32)
    h0 = np.asarray(inputs["hidden_state"], dtype=np.float32).reshape(H)
    c0 = np.asarray(inputs["cell_state"], dtype=np.float32).reshape(H)
    enc = np.asarray(inputs["encoder_outputs"], dtype=np.float32)
    emb = _bf16(inputs["emb"])
    aW = np.asarray(inputs["attn_W"], dtype=np.float32)
    abv = np.asarray(inputs["attn_b"], dtype=np.float32)
    cWm = np.asarray(inputs["comb_W"], dtype=np.float32)
    cbv = np.asarray(inputs["comb_b"], dtype=np.float32)
    wihm = np.asarray(inputs["W_ih"], dtype=np.float32)
    whhm = np.asarray(inputs["W_hh"], dtype=np.float32)
    bgv = np.asarray(inputs["b_ih"], dtype=np.float32) + np.asarray(
        inputs["b_hh"], dtype=np.float32
    )
    oWm = np.asarray(inputs["out_W"], dtype=np.float32)
    obv = np.asarray(inputs["out_b"], dtype=np.float32)

    h0col = h0.reshape(8, 128).T
    c0col = c0.reshape(8, 128).T
    bgcol = bgv.reshape(32, 128).T
    oWp = _bf16(_pack_k(np.ascontiguousarray(oWm.T), V))
    rowf = np.zeros((1, 541), dtype=np.float32)
    rowf[0, 512:541] = obv

    in_maps = []
    for i in range(NCORES):
        aWp = _bf16(_pack_k(np.ascontiguousarray(aW[i * LSH : (i + 1) * LSH, :].T), LSH))
        encp = _bf16(_pack_k(enc[i * LSH : (i + 1) * LSH, :], H))
        cWp = _bf16(_pack_k(np.ascontiguousarray(cWm[i * HSH : (i + 1) * HSH, :].T), HSH))
        wihp = _bf16(wihm[:, i * HSH : (i + 1) * HSH].T)   # [128, 4096]
        whhp = _bf16(whhm[:, i * HSH : (i + 1) * HSH].T)
        smf = np.concatenate(
            [
                c0col,
                cbv[i * HSH : (i + 1) * HSH].reshape(128, 1),
                bgcol,
            ],
            axis=1,
        ).astype(np.float32)
        smb = _bf16(
            np.concatenate(
                [h0col, h0[i * HSH : (i + 1) * HSH].reshape(128, 1)], axis=1
            )
        )
        m = {
            "tok": tok,
            "emb": emb,
            "smf": np.ascontiguousarray(smf),
            "rowf": rowf,
            "smb": smb,
            "abb": _bf16(abv[i * LSH : (i + 1) * LSH].reshape(1, LSH)),
            "cW": cWp,
            "oW": oWp,
        }
        for j in range(4):
            m[f"aW{j}"] = np.ascontiguousarray(aWp[:, j * 2048 : (j + 1) * 2048])
        for j in range(2):
            m[f"enc{j}"] = np.ascontiguousarray(encp[:, j * 2048 : (j + 1) * 2048])
            m[f"wih{j}"] = np.ascontiguousarray(wihp[:, j * 2048 : (j + 1) * 2048])
            m[f"whh{j}"] = np.ascontiguousarray(whhp[:, j * 2048 : (j + 1) * 2048])
        in_maps.append(m)
    return in_maps


def unpack_outputs(res0):
    log_probs = np.asarray(res0["logp"], dtype=np.float32).reshape(1, V)
    h_new = np.ascontiguousarray(
        np.asarray(res0["hout"], dtype=np.float32).T
    ).reshape(1, 1, H)
    c_new = np.ascontiguousarray(
        np.asarray(res0["cout"], dtype=np.float32).T
    ).reshape(1, 1, H)
    attn_w = np.asarray(res0["awout"], dtype=np.float32).reshape(1, L)
    return (log_probs, h_new, c_new, attn_w)


def get_nc():
    if "nc" not in _CACHE:
        _CACHE["nc"] = _build_nc()
    return _CACHE["nc"]


def kernel(**inputs):
    from concourse import bass_utils

    nc = get_nc()
    in_maps = prep_in_maps(inputs)
    out = bass_utils.run_bass_kernel_spmd(nc, in_maps, core_ids=list(range(NCORES)))
    return unpack_outputs(out.results[0])


if __name__ == "__main__":
    nc = _build_nc()
    print("built ok; instructions:", sum(len(bb.instructions) for bb in nc.main_func.blocks))


# revision 18
# speedup vs baseline: 1.7173x; 1.0130x over previous
"""AttentionDecoderRNN single-step decode on 8 TRN2 NeuronCores.

Strategy (tensor-parallel matvec, 1 AllGather + 1 AllReduce):
  - attn_W sharded over output rows (512 logits/core); encoder_outputs
    sharded over the same rows, so each core computes an *unnormalized*
    attention-value partial u_i = sum_j exp(logit_ij - m_i) * enc_i[j,:]
    plus local stats (m_i, s_i) -- flash-attention style.
  - AllGather #1 carries {u_i, s_i, m_i, logits_i}; every core then
    reconstructs the exact global softmax via c_i = exp(m_i - M)/S and
    gets attn_applied plus the full attn_weights output.
  - comb_W sharded over output rows (128/core) -> local x slice (relu).
  - W_ih / W_hh sharded over input columns matching the x / h0 slice
    each core owns -> partial LSTM gates (4096).
  - AllReduce #2 (add) sums the gate partials; every core runs the LSTM
    elementwise math + output layer + log_softmax redundantly.

Performance notes (from trace analysis of the f32 v1):
  - fp32 matmuls lower to 2x(LDWEIGHTS+MATMUL) at ~430ns per 128x128
    tile -> PE consumes weights at only ~149GB/s.  v2 stores all big
    weights in bf16 (halves DMA bytes; rel err ~2.7e-3, gate is 2e-2)
    and flips the two big matvecs so the weight matrix is the *moving*
    operand: one matmul covers [128K x 512N] (16 instructions for the
    whole attention logit matvec instead of 64).
  - A single HWDGE ring sustains only ~146GB/s, so the weight streams
    are spread across the vector / tensor / gpsimd rings.  The sync
    ring is kept free for latency-critical mid-kernel DMAs (collective
    bounce traffic); the scalar ring is avoided entirely (it carries
    ~39k 4-byte event-semaphore packets and showed 15-30us completion
    latencies).
  - Gate partials go through AllReduce (CCE adds) instead of AllGather
    + on-chip sum: the strided 8-rank interleaved readback cost ~36us.
  - Activation tables (Exp/Sigmoid/Tanh/Ln) are warmed with dummy ops
    at t=0 so the 1.3us ACT_TABLE_LOADs stay off the critical path.
"""

import numpy as np

NCORES = 8
V = 29
H = 1024
L = 4096
LSH = L // NCORES    # 512 logits per core
HSH = H // NCORES    # 128 hidden per core
CTR = 1544           # contrib floats: 1024 u + 1 s + 1 m + 512 logits + 6 pad

_CACHE = {}


def _bf16(x):
    import ml_dtypes

    return np.ascontiguousarray(
        np.asarray(x, dtype=np.float32).astype(ml_dtypes.bfloat16)
    )


def _pack_k(mat, ncols):
    """[K, ncols] (K = 128*nk) -> [128, nk*ncols]; block kt lands at
    columns [kt*ncols, (kt+1)*ncols) so out[p, kt*ncols+c] = mat[kt*128+p, c]."""
    K = mat.shape[0]
    nk = K // 128
    return np.ascontiguousarray(
        mat.reshape(nk, 128, ncols).transpose(1, 0, 2).reshape(128, nk * ncols)
    )


def _build_nc():
    import concourse.bacc as bacc
    import concourse.mybir as mybir
    from concourse import tile

    F32 = mybir.dt.float32
    BF16 = mybir.dt.bfloat16
    I32 = mybir.dt.int32
    AF = mybir.ActivationFunctionType
    ALU = mybir.AluOpType
    AX = mybir.AxisListType

    nc = bacc.Bacc(
        "TRN2", target_bir_lowering=False, debug=False, num_devices=NCORES
    )

    def inp(name, shape, dt=F32):
        return nc.dram_tensor(name, shape, dt, kind="ExternalInput")

    tok_d = inp("tok", [1, 1], I32)
    emb_d = inp("emb", [V, H], BF16)
    smf_d = inp("smf", [128, 41])            # c0col | cb | bg   (f32)
    rowf_d = inp("rowf", [1, 541])           # ob pad'd row      (f32)
    smb_d = inp("smb", [128, 9], BF16)       # h0col | h0slice   (bf16)
    abb_d = inp("abb", [1, LSH], BF16)       # attn_b slice row  (bf16)
    aW_d = [inp(f"aW{j}", [128, 2048], BF16) for j in range(4)]
    enc_d = [inp(f"enc{j}", [128, 2048], BF16) for j in range(2)]
    cW_d = inp("cW", [128, 2048], BF16)
    wih_d = [inp(f"wih{j}", [128, 2048], BF16) for j in range(2)]
    whh_d = [inp(f"whh{j}", [128, 2048], BF16) for j in range(2)]
    oW_d = inp("oW", [128, 8 * V], BF16)

    o_lp = nc.dram_tensor("logp", [1, V], F32, kind="ExternalOutput")
    o_h = nc.dram_tensor("hout", [128, 8], F32, kind="ExternalOutput")
    o_c = nc.dram_tensor("cout", [128, 8], F32, kind="ExternalOutput")
    o_aw = nc.dram_tensor("awout", [NCORES, LSH], F32, kind="ExternalOutput")

    iota_d = nc.inline_tensor(
        np.arange(V, dtype=np.float32).reshape(V, 1), name="iota29"
    )
    id8_d = nc.inline_tensor(np.eye(8, dtype=np.float32), name="ident8")


    RG = [list(range(NCORES))]

    with tile.TileContext(nc) as tc:
        with (
            tc.tile_pool(name="wts", bufs=1) as wp,
            tc.tile_pool(name="sml", bufs=1) as sp,
            tc.tile_pool(name="psum", bufs=4, space="PSUM") as pp,
            tc.tile_pool(name="psumu", bufs=1, space="PSUM") as ppu,
            tc.tile_pool(name="psums", bufs=1, space="PSUM") as pps,
            tc.tile_pool(name="dram", bufs=1, space="DRAM") as dp,
        ):
            # ---------- small input loads (sync ring, all tiny) ----------
            emb_sb = wp.tile([V, H], BF16, tag="emb")
            nc.sync.dma_start(emb_sb[:], emb_d.ap())
            smf = wp.tile([128, 41], F32, tag="smf")
            nc.sync.dma_start(smf[:], smf_d.ap())
            rowf = wp.tile([1, 541], F32, tag="rowf")
            nc.sync.dma_start(rowf[:], rowf_d.ap())
            smb = wp.tile([128, 9], BF16, tag="smb")
            nc.sync.dma_start(smb[:], smb_d.ap())
            abb = wp.tile([1, LSH], BF16, tag="abb")
            nc.sync.dma_start(abb[:], abb_d.ap())
            iota = wp.tile([V, 1], F32, tag="iota")
            nc.sync.dma_start(iota[:], iota_d.ap())
            id8 = wp.tile([8, 8], F32, tag="id8")
            nc.sync.dma_start(id8[:], id8_d.ap())
            tok_i = sp.tile([1, 1], I32, tag="tok_i")
            nc.sync.dma_start(tok_i[:], tok_d.ap())
            oW = wp.tile([128, 8 * V], BF16, tag="oW")
            nc.sync.dma_start(oW[:], oW_d.ap())

            c0c = smf[:, 0:8]
            cb = smf[:, 8:9]
            bg = smf[:, 9:41]
            ob = rowf[:, 512:541]
            h0c = smb[:, 0:8]
            h0s = smb[:, 8:9]

            # ---------- big weight streams, spread across rings ----------
            aW = [wp.tile([128, 2048], BF16, tag=f"aW{j}", name=f"aW{j}_sb") for j in range(4)]
            encs = [wp.tile([128, 2048], BF16, tag=f"enc{j}", name=f"enc{j}_sb") for j in range(2)]
            cW = wp.tile([128, 2048], BF16, tag="cW")
            whh = [wp.tile([128, 2048], BF16, tag=f"whh{j}", name=f"whh{j}_sb") for j in range(2)]
            wih = [wp.tile([128, 2048], BF16, tag=f"wih{j}", name=f"wih{j}_sb") for j in range(2)]
            # gpsimd SWDGE ring (measured ~270GB/s, clean completions):
            # the dependency-chain weights, in consumption order.  The
            # sync HWDGE ring carries the event-semaphore flood, which
            # delays completion semaphores by up to ~33us -- so only the
            # LSTM weights (whose consumers wait on data sems and run
            # late anyway) plus the small preloads go there.
            for j in range(4):
                nc.gpsimd.dma_start(aW[j][:], aW_d[j].ap())
            for j in range(2):
                nc.gpsimd.dma_start(encs[j][:], enc_d[j].ap())
            nc.gpsimd.dma_start(cW[:], cW_d.ap())
            for j in range(2):
                nc.sync.dma_start(whh[j][:], whh_d[j].ap())
            for j in range(2):
                nc.sync.dma_start(wih[j][:], wih_d[j].ap())

            # ---------- constants + activation-table warmup ----------
            ones29 = sp.tile([1, V], F32, tag="ones29")
            nc.vector.memset(ones29[:], 1.0)
            ones8r = sp.tile([1, 8], F32, tag="ones8r")
            nc.vector.memset(ones8r[:], 1.0)
            ones8c = sp.tile([8, 1], F32, tag="ones8c")
            nc.vector.memset(ones8c[:], 1.0)
            one_bf = sp.tile([1, 1], BF16, tag="one_bf")
            nc.vector.memset(one_bf[:], 1.0)
            zpad = sp.tile([1, 6], F32, tag="zpad")
            nc.vector.memset(zpad[:], 0.0)
            warm = sp.tile([1, 4], F32, tag="warm")
            nc.vector.memset(warm[:], 0.5)
            for fn in (AF.Exp, AF.Sigmoid, AF.Tanh, AF.Ln):
                nc.scalar.activation(warm[:], warm[:], fn)

            # ---------- embedded = emb[tok] via on-device one-hot ----------
            tok_f = sp.tile([1, 1], F32, tag="tok_f")
            nc.vector.tensor_copy(tok_f[:], tok_i[:])
            ps_tb = pps.tile([V, 1], F32, tag="pss")
            nc.tensor.matmul(ps_tb[:], ones29[:], tok_f[:], start=True, stop=True)
            tokb = sp.tile([V, 1], F32, tag="tokb")
            nc.vector.tensor_copy(tokb[:], ps_tb[:])
            onehot = sp.tile([V, 1], F32, tag="onehot")
            nc.vector.tensor_tensor(onehot[:], iota[:], tokb[:], ALU.is_equal)
            ohb = sp.tile([V, 1], BF16, tag="ohb")
            nc.vector.tensor_copy(ohb[:], onehot[:])
            ps_emb = pp.tile([128, 8], F32, tag="ps")
            for ht in range(8):
                nc.tensor.matmul(
                    ps_emb[:, ht : ht + 1],
                    emb_sb[:, ht * 128 : (ht + 1) * 128],
                    ohb[:],
                    start=True,
                    stop=True,
                )
            cat2 = sp.tile([128, 16], BF16, tag="cat2")
            nc.vector.tensor_copy(cat2[:, 0:8], ps_emb[:])

            # ---------- attention logits (row form, weights moving) ------
            # logits[l] = sum_d cat1[d] * attn_W[l, d] + attn_b[l]
            ps_lg = ppu.tile([1, LSH], F32, tag="pslg")
            for kt in range(16):
                vec = cat2[:, kt : kt + 1] if kt < 8 else h0c[:, kt - 8 : kt - 7]
                nc.tensor.matmul(
                    ps_lg[:],
                    vec,
                    aW[kt // 4][:, (kt % 4) * 512 : (kt % 4 + 1) * 512],
                    start=(kt == 0),
                    stop=False,
                )
            nc.tensor.matmul(ps_lg[:], one_bf[:], abb[:], start=False, stop=True)

            # local softmax stats on the row
            negm = sp.tile([1, 1], F32, tag="negm")
            nc.vector.tensor_reduce(negm[:], ps_lg[:], AX.X, ALU.max, negate=True)
            expr = sp.tile([1, LSH], F32, tag="expr")
            s_row = sp.tile([1, 1], F32, tag="s_row")
            nc.scalar.activation(
                expr[:], ps_lg[:], AF.Exp, bias=negm[:], accum_out=s_row[:]
            )
            sm = sp.tile([1, 2], F32, tag="sm")
            nc.vector.tensor_copy(sm[:, 0:1], s_row[:])
            nc.vector.tensor_scalar_mul(sm[:, 1:2], negm[:], -1.0)

            # transpose exp weights row -> column [128, 4] (f32: PSUM
            # writes must stay 4-byte aligned; cast to bf16 on the copy out)
            ps_tc = pp.tile([128, 4], F32, tag="ps")
            for j in range(4):
                nc.tensor.transpose(
                    ps_tc[:, j : j + 1], expr[:, j * 128 : (j + 1) * 128],
                    ones29[:, 0:1]
                )
            expc = sp.tile([128, 4], BF16, tag="expc")
            nc.vector.tensor_copy(expc[:], ps_tc[:])

            # ---------- u = expw @ enc_shard (row form) ----------
            ps_u = ppu.tile([1, H], F32, tag="psu")
            for nt in range(2):
                for kt in range(4):
                    nc.tensor.matmul(
                        ps_u[:, nt * 512 : (nt + 1) * 512],
                        expc[:, kt : kt + 1],
                        encs[kt // 2][:, (kt % 2) * 1024 + nt * 512 : (kt % 2) * 1024 + nt * 512 + 512],
                        start=(kt == 0),
                        stop=(kt == 3),
                    )

            # ---------- AllGather #1: {u, s, m, logits} ----------
            # (DMA cannot read PSUM -- stage the row results in SBUF,
            # splitting the single-partition copies across DVE and ACT)
            u_row = sp.tile([1, H], F32, tag="u_row")
            nc.vector.tensor_copy(u_row[:, 0:512], ps_u[:, 0:512])
            nc.scalar.copy(u_row[:, 512:1024], ps_u[:, 512:1024])
            lgr = sp.tile([1, LSH], F32, tag="lgr")
            nc.vector.tensor_copy(lgr[:], ps_lg[:])
            contrib = dp.tile([CTR], F32, tag="contrib")
            gath1 = dp.tile([NCORES, CTR], F32, tag="gath1")
            nc.gpsimd.dma_start(
                contrib[0:1024].rearrange("(a b) -> a b", a=1), u_row[:]
            )
            nc.gpsimd.dma_start(
                contrib[1024:1026].rearrange("(a b) -> a b", a=1), sm[:]
            )
            nc.gpsimd.dma_start(
                contrib[1026:1538].rearrange("(a b) -> a b", a=1), lgr[:]
            )
            nc.gpsimd.dma_start(
                contrib[1538:1544].rearrange("(a b) -> a b", a=1), zpad[:]
            )
            nc.gpsimd.collective_compute(
                "AllGather",
                ALU.bypass,
                replica_groups=RG,
                ins=[contrib[:].opt()],
                outs=[gath1[:].opt()],
            )
            g1 = sp.tile([NCORES, CTR], F32, tag="g1")
            nc.gpsimd.dma_start(g1[:], gath1[:])

            # ---------- softmax rescale coefficients c_i = exp(m_i-M)/S --
            m_col = g1[0:8, 1025:1026]
            s_col = g1[0:8, 1024:1025]
            ps_t8 = pps.tile([1, 8], F32, tag="pss")
            nc.tensor.transpose(ps_t8[:], m_col, id8[:])
            mrow = sp.tile([1, 8], F32, tag="mrow")
            nc.vector.tensor_copy(mrow[:], ps_t8[:])
            negM = sp.tile([1, 1], F32, tag="negM")
            nc.vector.tensor_reduce(negM[:], mrow[:], AX.X, ALU.max, negate=True)
            ps_nM = pps.tile([8, 1], F32, tag="pss")
            nc.tensor.matmul(ps_nM[:], ones8r[:], negM[:], start=True, stop=True)
            negMb = sp.tile([8, 1], F32, tag="negMb")
            nc.vector.tensor_copy(negMb[:], ps_nM[:])
            d_col = sp.tile([8, 1], F32, tag="d_col")
            nc.scalar.activation(d_col[:], m_col, AF.Exp, bias=negMb[:])
            sd_col = sp.tile([8, 1], F32, tag="sd_col")
            nc.vector.tensor_mul(sd_col[:], d_col[:], s_col)
            ps_S = pps.tile([1, 1], F32, tag="pss")
            nc.tensor.matmul(ps_S[:], sd_col[:], ones8c[:], start=True, stop=True)
            S_sb = sp.tile([1, 1], F32, tag="S_sb")
            nc.vector.tensor_copy(S_sb[:], ps_S[:])
            rS = sp.tile([1, 1], F32, tag="rS")
            nc.vector.reciprocal(rS[:], S_sb[:])
            ps_rS = pps.tile([8, 1], F32, tag="pss")
            nc.tensor.matmul(ps_rS[:], ones8r[:], rS[:], start=True, stop=True)
            rSb = sp.tile([8, 1], F32, tag="rSb")
            nc.vector.tensor_copy(rSb[:], ps_rS[:])
            c_col = sp.tile([8, 1], F32, tag="c_col")
            nc.vector.tensor_mul(c_col[:], d_col[:], rSb[:])

            # ---------- attn_applied = sum_i c_i * u_i ----------
            ps_aa = pp.tile([128, 8], F32, tag="ps")
            for ht in range(8):
                nc.tensor.matmul(
                    ps_aa[:, ht : ht + 1],
                    g1[0:8, ht * 128 : (ht + 1) * 128],
                    c_col[:],
                    start=True,
                    stop=True,
                )
            nc.vector.tensor_copy(cat2[:, 8:16], ps_aa[:])

            # ---------- attn_weights output (full, exact softmax) --------
            negm8 = sp.tile([8, 1], F32, tag="negm8")
            nc.vector.tensor_scalar_mul(negm8[:], m_col, -1.0)
            aw = sp.tile([8, LSH], F32, tag="aw")
            nc.scalar.activation(aw[:], g1[0:8, 1026:1538], AF.Exp, bias=negm8[:])
            nc.vector.tensor_scalar_mul(aw[:], aw[:], c_col[:])
            nc.sync.dma_start(o_aw.ap(), aw[:])

            # ---------- x slice = relu(comb_W_rows @ cat2 + b) ----------
            ps_x = pp.tile([128, 1], F32, tag="ps")
            for kt in range(16):
                nc.tensor.matmul(
                    ps_x[:],
                    cW[:, kt * 128 : (kt + 1) * 128],
                    cat2[:, kt : kt + 1],
                    start=(kt == 0),
                    stop=(kt == 15),
                )
            x_col = sp.tile([128, 1], BF16, tag="x_col")
            nc.scalar.activation(x_col[:], ps_x[:], AF.Relu, bias=cb)

            # ---------- partial LSTM gates ----------
            # W_hh @ h0_slice runs early (h0 is an input); W_ih @ x_slice
            # waits for x.  Separate PSUM tiles keep the accumulation
            # groups bank-sequential; summed on the way to DRAM.
            ps_gh = pp.tile([128, 32], F32, tag="ps")
            for j in range(2):
                for mt in range(16):
                    col = j * 16 + mt
                    nc.tensor.matmul(
                        ps_gh[:, col : col + 1],
                        whh[j][:, mt * 128 : (mt + 1) * 128],
                        h0s,
                        start=True,
                        stop=True,
                    )
            ghh_sb = sp.tile([128, 32], F32, tag="ghh_sb")
            nc.vector.tensor_copy(ghh_sb[:], ps_gh[:])
            ps_g = pp.tile([128, 32], F32, tag="ps")
            for j in range(2):
                for mt in range(16):
                    col = j * 16 + mt
                    nc.tensor.matmul(
                        ps_g[:, col : col + 1],
                        wih[j][:, mt * 128 : (mt + 1) * 128],
                        x_col[:],
                        start=True,
                        stop=True,
                    )
            gp_sb = sp.tile([128, 32], F32, tag="gp_sb")
            nc.vector.tensor_add(gp_sb[:], ps_g[:], ghh_sb[:])

            # ---------- AllReduce #2: sum gate partials across cores -----
            contrib2 = dp.tile([4096], F32, tag="contrib2")
            gsum = dp.tile([4096], F32, tag="gsum")
            nc.gpsimd.dma_start(
                contrib2[:].rearrange("(f p) -> p f", p=128), gp_sb[:]
            )
            nc.gpsimd.collective_compute(
                "AllReduce",
                ALU.add,
                replica_groups=RG,
                ins=[contrib2[:].opt()],
                outs=[gsum[:].opt()],
            )
            g2c = sp.tile([128, 32], F32, tag="g2c")
            nc.gpsimd.dma_start(g2c[:], gsum[:].rearrange("(f p) -> p f", p=128))

            gates = sp.tile([128, 32], F32, tag="gates")
            nc.vector.tensor_add(gates[:], g2c[:], bg)

            # ---------- LSTM elementwise (gate order i, f, g, o) ----------
            sig = sp.tile([128, 24], F32, tag="sig")
            nc.scalar.activation(sig[:, 0:8], gates[:, 0:8], AF.Sigmoid)
            nc.scalar.activation(sig[:, 8:16], gates[:, 8:16], AF.Sigmoid)
            nc.scalar.activation(sig[:, 16:24], gates[:, 24:32], AF.Sigmoid)
            gt = sp.tile([128, 8], F32, tag="gt")
            nc.scalar.activation(gt[:], gates[:, 16:24], AF.Tanh)
            cnew = sp.tile([128, 8], F32, tag="cnew")
            t1 = sp.tile([128, 8], F32, tag="t1")
            nc.vector.tensor_mul(cnew[:], sig[:, 8:16], c0c)
            nc.vector.tensor_mul(t1[:], sig[:, 0:8], gt[:])
            nc.vector.tensor_add(cnew[:], cnew[:], t1[:])
            tcn = sp.tile([128, 8], F32, tag="tcn")
            nc.scalar.activation(tcn[:], cnew[:], AF.Tanh)
            hnew = sp.tile([128, 8], F32, tag="hnew")
            nc.vector.tensor_mul(hnew[:], sig[:, 16:24], tcn[:])
            nc.sync.dma_start(o_h.ap(), hnew[:])
            nc.sync.dma_start(o_c.ap(), cnew[:])

            # ---------- output layer + log_softmax ----------
            h_bf = sp.tile([128, 8], BF16, tag="h_bf")
            nc.vector.tensor_copy(h_bf[:], hnew[:])
            ps_o = pps.tile([1, V], F32, tag="pss")
            for kt in range(8):
                nc.tensor.matmul(
                    ps_o[:],
                    h_bf[:, kt : kt + 1],
                    oW[:, kt * V : (kt + 1) * V],
                    start=(kt == 0),
                    stop=(kt == 7),
                )
            orow = sp.tile([1, V], F32, tag="orow")
            nc.vector.tensor_add(orow[:], ps_o[:], ob)
            negmx = sp.tile([1, 1], F32, tag="negmx")
            nc.vector.tensor_reduce(negmx[:], orow[:], AX.X, ALU.max, negate=True)
            erow = sp.tile([1, V], F32, tag="erow")
            sume = sp.tile([1, 1], F32, tag="sume")
            nc.scalar.activation(
                erow[:], orow[:], AF.Exp, bias=negmx[:], accum_out=sume[:]
            )
            lse = sp.tile([1, 1], F32, tag="lse")
            nc.scalar.activation(lse[:], sume[:], AF.Ln)
            lp = sp.tile([1, V], F32, tag="lp")
            nc.vector.tensor_scalar(
                lp[:], orow[:], negmx[:], lse[:], ALU.add, ALU.subtract
            )
            nc.sync.dma_start(o_lp.ap(), lp[:])

    nc.finalize()  # Bacc.finalize = compile passes (reg alloc etc) + freeze
    return nc


def prep_in_maps(inputs):
    tok = np.asarray(inputs["input_tok"]).reshape(1, 1).astype(np.int32)
    h0 = np.asarray(inputs["hidden_state"], dtype=np.float32).reshape(H)
    c0 = np.asarray(inputs["cell_state"], dtype=np.float32).reshape(H)
    enc = np.asarray(inputs["encoder_outputs"], dtype=np.float32)
    emb = _bf16(inputs["emb"])
    aW = np.asarray(inputs["attn_W"], dtype=np.float32)
    abv = np.asarray(inputs["attn_b"], dtype=np.float32)
    cWm = np.asarray(inputs["comb_W"], dtype=np.float32)
    cbv = np.asarray(inputs["comb_b"], dtype=np.float32)
    wihm = np.asarray(inputs["W_ih"], dtype=np.float32)
    whhm = np.asarray(inputs["W_hh"], dtype=np.float32)
    bgv = np.asarray(inputs["b_ih"], dtype=np.float32) + np.asarray(
        inputs["b_hh"], dtype=np.float32
    )
    oWm = np.asarray(inputs["out_W"], dtype=np.float32)
    obv = np.asarray(inputs["out_b"], dtype=np.float32)

    h0col = h0.reshape(8, 128).T
    c0col = c0.reshape(8, 128).T
    bgcol = bgv.reshape(32, 128).T
    oWp = _bf16(_pack_k(np.ascontiguousarray(oWm.T), V))
    rowf = np.zeros((1, 541), dtype=np.float32)
    rowf[0, 512:541] = obv

    in_maps = []
    for i in range(NCORES):
        aWp = _bf16(_pack_k(np.ascontiguousarray(aW[i * LSH : (i + 1) * LSH, :].T), LSH))
        encp = _bf16(_pack_k(enc[i * LSH : (i + 1) * LSH, :], H))
        cWp = _bf16(_pack_k(np.ascontiguousarray(cWm[i * HSH : (i + 1) * HSH, :].T), HSH))
        wihp = _bf16(wihm[:, i * HSH : (i + 1) * HSH].T)   # [128, 4096]
        whhp = _bf16(whhm[:, i * HSH : (i + 1) * HSH].T)
        smf = np.concatenate(
            [
                c0col,
                cbv[i * HSH : (i + 1) * HSH].reshape(128, 1),
                bgcol,
            ],
            axis=1,
        ).astype(np.float32)
        smb = _bf16(
            np.concatenate(
                [h0col, h0[i * HSH : (i + 1) * HSH].reshape(128, 1)], axis=1
            )
        )
        m = {
            "tok": tok,
            "emb": emb,
            "smf": np.ascontiguousarray(smf),
            "rowf": rowf,
            "smb": smb,
            "abb": _bf16(abv[i * LSH : (i + 1) * LSH].reshape(1, LSH)),
            "cW": cWp,
            "oW": oWp,
        }
        for j in range(4):
            m[f"aW{j}"] = np.ascontiguousarray(aWp[:, j * 2048 : (j + 1) * 2048])
        for j in range(2):
            m[f"enc{j}"] = np.ascontiguousarray(encp[:, j * 2048 : (j + 1) * 2048])
            m[f"wih{j}"] = np.ascontiguousarray(wihp[:, j * 2048 : (j + 1) * 2048])
            m[f"whh{j}"] = np.ascontiguousarray(whhp[:, j * 2048 : (j + 1) * 2048])
        in_maps.append(m)
    return in_maps


def unpack_outputs(res0):
    log_probs = np.asarray(res0["logp"], dtype=np.float32).reshape(1, V)
    h_new = np.ascontiguousarray(
        np.asarray(res0["hout"], dtype=np.float32).T
    ).reshape(1, 1, H)
    c_new = np.ascontiguousarray(
        np.asarray(res0["cout"], dtype=np.float32).T
    ).reshape(1, 1, H)
    attn_w = np.asarray(res0["awout"], dtype=np.float32).reshape(1, L)
    return (log_probs, h_new, c_new, attn_w)


def get_nc():
    if "nc" not in _CACHE:
        _CACHE["nc"] = _build_nc()
    return _CACHE["nc"]


def kernel(**inputs):
    from concourse import bass_utils

    nc = get_nc()
    in_maps = prep_in_maps(inputs)
    out = bass_utils.run_bass_kernel_spmd(nc, in_maps, core_ids=list(range(NCORES)))
    return unpack_outputs(out.results[0])


if __name__ == "__main__":
    nc = _build_nc()
    print("built ok; instructions:", sum(len(bb.instructions) for bb in nc.main_func.blocks))


# revision 19
# speedup vs baseline: 2.1816x; 1.2703x over previous
"""AttentionDecoderRNN single-step decode on 8 TRN2 NeuronCores.

Strategy (tensor-parallel matvec, 1 AllGather + 1 AllReduce):
  - attn_W sharded over output rows (512 logits/core); encoder_outputs
    sharded over the same rows, so each core computes an *unnormalized*
    attention-value partial u_i = sum_j exp(logit_ij - m_i) * enc_i[j,:]
    plus local stats (m_i, s_i) -- flash-attention style.
  - AllGather #1 carries {u_i, s_i, m_i, logits_i}; every core then
    reconstructs the exact global softmax via c_i = exp(m_i - M)/S and
    gets attn_applied plus the full attn_weights output.
  - comb_W sharded over output rows (128/core) -> local x slice (relu).
  - W_ih / W_hh sharded over input columns matching the x / h0 slice
    each core owns -> partial LSTM gates (4096).
  - AllReduce #2 (add) sums the gate partials; every core runs the LSTM
    elementwise math + output layer + log_softmax redundantly.

Performance notes (from trace analysis of the f32 v1):
  - fp32 matmuls lower to 2x(LDWEIGHTS+MATMUL) at ~430ns per 128x128
    tile -> PE consumes weights at only ~149GB/s.  v2 stores all big
    weights in bf16 (halves DMA bytes; rel err ~2.7e-3, gate is 2e-2)
    and flips the two big matvecs so the weight matrix is the *moving*
    operand: one matmul covers [128K x 512N] (16 instructions for the
    whole attention logit matvec instead of 64).
  - A single HWDGE ring sustains only ~146GB/s, so the weight streams
    are spread across the vector / tensor / gpsimd rings.  The sync
    ring is kept free for latency-critical mid-kernel DMAs (collective
    bounce traffic); the scalar ring is avoided entirely (it carries
    ~39k 4-byte event-semaphore packets and showed 15-30us completion
    latencies).
  - Gate partials go through AllReduce (CCE adds) instead of AllGather
    + on-chip sum: the strided 8-rank interleaved readback cost ~36us.
  - Activation tables (Exp/Sigmoid/Tanh/Ln) are warmed with dummy ops
    at t=0 so the 1.3us ACT_TABLE_LOADs stay off the critical path.
"""

import numpy as np

NCORES = 8
V = 29
H = 1024
L = 4096
LSH = L // NCORES    # 512 logits per core
HSH = H // NCORES    # 128 hidden per core
CTR = 1544           # contrib floats: 1024 u + 1 s + 1 m + 512 logits + 6 pad

_CACHE = {}


def _bf16(x):
    import ml_dtypes

    return np.ascontiguousarray(
        np.asarray(x, dtype=np.float32).astype(ml_dtypes.bfloat16)
    )


def _pack_k(mat, ncols):
    """[K, ncols] (K = 128*nk) -> [128, nk*ncols]; block kt lands at
    columns [kt*ncols, (kt+1)*ncols) so out[p, kt*ncols+c] = mat[kt*128+p, c]."""
    K = mat.shape[0]
    nk = K // 128
    return np.ascontiguousarray(
        mat.reshape(nk, 128, ncols).transpose(1, 0, 2).reshape(128, nk * ncols)
    )


def _build_nc():
    import concourse.bacc as bacc
    import concourse.mybir as mybir
    from concourse import tile

    F32 = mybir.dt.float32
    BF16 = mybir.dt.bfloat16
    I32 = mybir.dt.int32
    AF = mybir.ActivationFunctionType
    ALU = mybir.AluOpType
    AX = mybir.AxisListType

    nc = bacc.Bacc(
        "TRN2", target_bir_lowering=False, debug=False, num_devices=NCORES
    )

    def inp(name, shape, dt=F32):
        return nc.dram_tensor(name, shape, dt, kind="ExternalInput")

    tok_d = inp("tok", [1, 1], I32)
    emb_d = inp("emb", [V, H], BF16)
    smf_d = inp("smf", [128, 41])            # c0col | cb | bg   (f32)
    rowf_d = inp("rowf", [1, 541])           # ob pad'd row      (f32)
    smb_d = inp("smb", [128, 9], BF16)       # h0col | h0slice   (bf16)
    abb_d = inp("abb", [1, LSH], BF16)       # attn_b slice row  (bf16)
    aW_d = [inp(f"aW{j}", [128, 2048], BF16) for j in range(4)]
    enc_d = [inp(f"enc{j}", [128, 2048], BF16) for j in range(2)]
    cW_d = inp("cW", [128, 2048], BF16)
    wih_d = [inp(f"wih{j}", [128, 2048], BF16) for j in range(2)]
    whh_d = [inp(f"whh{j}", [128, 2048], BF16) for j in range(2)]
    oW_d = inp("oW", [128, 8 * V], BF16)

    o_lp = nc.dram_tensor("logp", [1, V], F32, kind="ExternalOutput")
    o_h = nc.dram_tensor("hout", [128, 8], F32, kind="ExternalOutput")
    o_c = nc.dram_tensor("cout", [128, 8], F32, kind="ExternalOutput")
    o_aw = nc.dram_tensor("awout", [NCORES, LSH], F32, kind="ExternalOutput")

    iota_d = nc.inline_tensor(
        np.arange(V, dtype=np.float32).reshape(V, 1), name="iota29"
    )
    id8_d = nc.inline_tensor(np.eye(8, dtype=np.float32), name="ident8")
    id128_d = nc.inline_tensor(np.eye(128, dtype=np.float32), name="ident128")


    RG = [list(range(NCORES))]

    with tile.TileContext(nc) as tc:
        with (
            tc.tile_pool(name="wts", bufs=1) as wp,
            tc.tile_pool(name="sml", bufs=1) as sp,
            tc.tile_pool(name="psum", bufs=4, space="PSUM") as pp,
            tc.tile_pool(name="psumu", bufs=1, space="PSUM") as ppu,
            tc.tile_pool(name="psums", bufs=1, space="PSUM") as pps,
            tc.tile_pool(name="dram", bufs=1, space="DRAM") as dp,
        ):
            # ---------- small input loads (sync ring, all tiny) ----------
            emb_sb = wp.tile([V, H], BF16, tag="emb")
            nc.sync.dma_start(emb_sb[:], emb_d.ap())
            smf = wp.tile([128, 41], F32, tag="smf")
            nc.sync.dma_start(smf[:], smf_d.ap())
            rowf = wp.tile([1, 541], F32, tag="rowf")
            nc.sync.dma_start(rowf[:], rowf_d.ap())
            smb = wp.tile([128, 9], BF16, tag="smb")
            nc.sync.dma_start(smb[:], smb_d.ap())
            abb = wp.tile([1, LSH], BF16, tag="abb")
            nc.sync.dma_start(abb[:], abb_d.ap())
            iota = wp.tile([V, 1], F32, tag="iota")
            nc.sync.dma_start(iota[:], iota_d.ap())
            id8 = wp.tile([8, 8], F32, tag="id8")
            nc.sync.dma_start(id8[:], id8_d.ap())
            id128 = wp.tile([128, 128], F32, tag="id128")
            nc.sync.dma_start(id128[:], id128_d.ap())
            tok_i = sp.tile([1, 1], I32, tag="tok_i")
            nc.sync.dma_start(tok_i[:], tok_d.ap())
            oW = wp.tile([128, 8 * V], BF16, tag="oW")
            nc.sync.dma_start(oW[:], oW_d.ap())

            c0c = smf[:, 0:8]
            cb = smf[:, 8:9]
            bg = smf[:, 9:41]
            ob = rowf[:, 512:541]
            h0c = smb[:, 0:8]
            h0s = smb[:, 8:9]

            # ---------- big weight streams, spread across rings ----------
            aW = [wp.tile([128, 2048], BF16, tag=f"aW{j}", name=f"aW{j}_sb") for j in range(4)]
            encs = [wp.tile([128, 2048], BF16, tag=f"enc{j}", name=f"enc{j}_sb") for j in range(2)]
            cW = wp.tile([128, 2048], BF16, tag="cW")
            whh = [wp.tile([128, 2048], BF16, tag=f"whh{j}", name=f"whh{j}_sb") for j in range(2)]
            wih = [wp.tile([128, 2048], BF16, tag=f"wih{j}", name=f"wih{j}_sb") for j in range(2)]
            # gpsimd SWDGE ring (measured ~270GB/s, clean completions):
            # the dependency-chain weights, in consumption order.  The
            # sync HWDGE ring carries the event-semaphore flood, which
            # delays completion semaphores by up to ~33us -- so only the
            # LSTM weights (whose consumers wait on data sems and run
            # late anyway) plus the small preloads go there.
            for j in range(4):
                nc.gpsimd.dma_start(aW[j][:], aW_d[j].ap())
            for j in range(2):
                nc.gpsimd.dma_start(encs[j][:], enc_d[j].ap())
            nc.gpsimd.dma_start(cW[:], cW_d.ap())
            for j in range(2):
                nc.sync.dma_start(whh[j][:], whh_d[j].ap())
            for j in range(2):
                nc.sync.dma_start(wih[j][:], wih_d[j].ap())

            # ---------- constants + activation-table warmup ----------
            ones29 = sp.tile([1, V], F32, tag="ones29")
            nc.vector.memset(ones29[:], 1.0)
            ones8r = sp.tile([1, 8], F32, tag="ones8r")
            nc.vector.memset(ones8r[:], 1.0)
            ones8c = sp.tile([8, 1], F32, tag="ones8c")
            nc.vector.memset(ones8c[:], 1.0)
            one_bf = sp.tile([1, 1], BF16, tag="one_bf")
            nc.vector.memset(one_bf[:], 1.0)
            zpad = sp.tile([1, 6], F32, tag="zpad")
            nc.vector.memset(zpad[:], 0.0)
            warm = sp.tile([1, 4], F32, tag="warm")
            nc.vector.memset(warm[:], 0.5)
            for fn in (AF.Exp, AF.Sigmoid, AF.Tanh, AF.Ln):
                nc.scalar.activation(warm[:], warm[:], fn)

            # ---------- embedded = emb[tok] via on-device one-hot ----------
            tok_f = sp.tile([1, 1], F32, tag="tok_f")
            nc.vector.tensor_copy(tok_f[:], tok_i[:])
            ps_tb = pps.tile([V, 1], F32, tag="pss")
            nc.tensor.matmul(ps_tb[:], ones29[:], tok_f[:], start=True, stop=True)
            tokb = sp.tile([V, 1], F32, tag="tokb")
            nc.vector.tensor_copy(tokb[:], ps_tb[:])
            onehot = sp.tile([V, 1], F32, tag="onehot")
            nc.vector.tensor_tensor(onehot[:], iota[:], tokb[:], ALU.is_equal)
            ohb = sp.tile([V, 1], BF16, tag="ohb")
            nc.vector.tensor_copy(ohb[:], onehot[:])
            ps_emb = pp.tile([128, 8], F32, tag="ps")
            for ht in range(8):
                nc.tensor.matmul(
                    ps_emb[:, ht : ht + 1],
                    emb_sb[:, ht * 128 : (ht + 1) * 128],
                    ohb[:],
                    start=True,
                    stop=True,
                )
            cat2 = sp.tile([128, 16], BF16, tag="cat2")
            nc.vector.tensor_copy(cat2[:, 0:8], ps_emb[:])

            # ---------- attention logits (row form, weights moving) ------
            # logits[l] = sum_d cat1[d] * attn_W[l, d] + attn_b[l]
            ps_lg = ppu.tile([1, LSH], F32, tag="pslg")
            for kt in range(16):
                vec = cat2[:, kt : kt + 1] if kt < 8 else h0c[:, kt - 8 : kt - 7]
                nc.tensor.matmul(
                    ps_lg[:],
                    vec,
                    aW[kt // 4][:, (kt % 4) * 512 : (kt % 4 + 1) * 512],
                    start=(kt == 0),
                    stop=False,
                )
            nc.tensor.matmul(ps_lg[:], one_bf[:], abb[:], start=False, stop=True)

            # local softmax stats on the row
            negm = sp.tile([1, 1], F32, tag="negm")
            nc.vector.tensor_reduce(negm[:], ps_lg[:], AX.X, ALU.max, negate=True)
            expr = sp.tile([1, LSH], F32, tag="expr")
            s_row = sp.tile([1, 1], F32, tag="s_row")
            nc.scalar.activation(
                expr[:], ps_lg[:], AF.Exp, bias=negm[:], accum_out=s_row[:]
            )
            sm = sp.tile([1, 2], F32, tag="sm")
            nc.vector.tensor_copy(sm[:, 0:1], s_row[:])
            nc.vector.tensor_scalar_mul(sm[:, 1:2], negm[:], -1.0)

            # transpose exp weights row -> column [128, 4] (f32: PSUM
            # writes must stay 4-byte aligned; cast to bf16 on the copy out)
            ps_tc = pp.tile([128, 4], F32, tag="ps")
            for j in range(4):
                nc.tensor.transpose(
                    ps_tc[:, j : j + 1], expr[:, j * 128 : (j + 1) * 128],
                    ones29[:, 0:1]
                )
            expc = sp.tile([128, 4], BF16, tag="expc")
            nc.vector.tensor_copy(expc[:], ps_tc[:])

            # ---------- u = expw @ enc_shard (row form) ----------
            ps_u = ppu.tile([1, H], F32, tag="psu")
            for nt in range(2):
                for kt in range(4):
                    nc.tensor.matmul(
                        ps_u[:, nt * 512 : (nt + 1) * 512],
                        expc[:, kt : kt + 1],
                        encs[kt // 2][:, (kt % 2) * 1024 + nt * 512 : (kt % 2) * 1024 + nt * 512 + 512],
                        start=(kt == 0),
                        stop=(kt == 3),
                    )

            # ---------- AllGather #1: {u, s, m, logits} ----------
            # Assemble the whole 1544-float payload in one SBUF row so the
            # bounce write is a single-descriptor DMA (SWDGE descriptor
            # emission costs ~0.7us each; a strided write would need 32).
            crow = sp.tile([1, CTR], F32, tag="crow")
            nc.vector.memset(crow[:, 1538:1544], 0.0)
            nc.vector.tensor_copy(crow[:, 0:512], ps_u[:, 0:512])
            nc.scalar.copy(crow[:, 512:1024], ps_u[:, 512:1024])
            nc.vector.tensor_copy(crow[:, 1024:1026], sm[:])
            nc.vector.tensor_copy(crow[:, 1026:1538], ps_lg[:])
            contrib = dp.tile([CTR], F32, tag="contrib")
            gath1 = dp.tile([NCORES, CTR], F32, tag="gath1")
            nc.gpsimd.dma_start(
                contrib[:].rearrange("(a b) -> a b", a=1), crow[:]
            )
            nc.gpsimd.collective_compute(
                "AllGather",
                ALU.bypass,
                replica_groups=RG,
                ins=[contrib[:].opt()],
                outs=[gath1[:].opt()],
            )
            g1 = sp.tile([NCORES, CTR], F32, tag="g1")
            nc.gpsimd.dma_start(g1[:], gath1[:])

            # ---------- softmax rescale coefficients c_i = exp(m_i-M)/S --
            m_col = g1[0:8, 1025:1026]
            s_col = g1[0:8, 1024:1025]
            ps_t8 = pps.tile([1, 8], F32, tag="pss")
            nc.tensor.transpose(ps_t8[:], m_col, id8[:])
            mrow = sp.tile([1, 8], F32, tag="mrow")
            nc.vector.tensor_copy(mrow[:], ps_t8[:])
            negM = sp.tile([1, 1], F32, tag="negM")
            nc.vector.tensor_reduce(negM[:], mrow[:], AX.X, ALU.max, negate=True)
            ps_nM = pps.tile([8, 1], F32, tag="pss")
            nc.tensor.matmul(ps_nM[:], ones8r[:], negM[:], start=True, stop=True)
            negMb = sp.tile([8, 1], F32, tag="negMb")
            nc.vector.tensor_copy(negMb[:], ps_nM[:])
            d_col = sp.tile([8, 1], F32, tag="d_col")
            nc.scalar.activation(d_col[:], m_col, AF.Exp, bias=negMb[:])
            sd_col = sp.tile([8, 1], F32, tag="sd_col")
            nc.vector.tensor_mul(sd_col[:], d_col[:], s_col)
            ps_S = pps.tile([1, 1], F32, tag="pss")
            nc.tensor.matmul(ps_S[:], sd_col[:], ones8c[:], start=True, stop=True)
            S_sb = sp.tile([1, 1], F32, tag="S_sb")
            nc.vector.tensor_copy(S_sb[:], ps_S[:])
            rS = sp.tile([1, 1], F32, tag="rS")
            nc.vector.reciprocal(rS[:], S_sb[:])
            ps_rS = pps.tile([8, 1], F32, tag="pss")
            nc.tensor.matmul(ps_rS[:], ones8r[:], rS[:], start=True, stop=True)
            rSb = sp.tile([8, 1], F32, tag="rSb")
            nc.vector.tensor_copy(rSb[:], ps_rS[:])
            c_col = sp.tile([8, 1], F32, tag="c_col")
            nc.vector.tensor_mul(c_col[:], d_col[:], rSb[:])

            # ---------- attn_applied = sum_i c_i * u_i ----------
            ps_aa = pp.tile([128, 8], F32, tag="ps")
            for ht in range(8):
                nc.tensor.matmul(
                    ps_aa[:, ht : ht + 1],
                    g1[0:8, ht * 128 : (ht + 1) * 128],
                    c_col[:],
                    start=True,
                    stop=True,
                )
            nc.vector.tensor_copy(cat2[:, 8:16], ps_aa[:])

            # ---------- attn_weights output (full, exact softmax) --------
            negm8 = sp.tile([8, 1], F32, tag="negm8")
            nc.vector.tensor_scalar_mul(negm8[:], m_col, -1.0)
            aw = sp.tile([8, LSH], F32, tag="aw")
            nc.scalar.activation(aw[:], g1[0:8, 1026:1538], AF.Exp, bias=negm8[:])
            nc.vector.tensor_scalar_mul(aw[:], aw[:], c_col[:])
            nc.sync.dma_start(o_aw.ap(), aw[:])

            # ---------- x slice = relu(comb_W_rows @ cat2 + b) ----------
            ps_x = pp.tile([128, 1], F32, tag="ps")
            for kt in range(16):
                nc.tensor.matmul(
                    ps_x[:],
                    cW[:, kt * 128 : (kt + 1) * 128],
                    cat2[:, kt : kt + 1],
                    start=(kt == 0),
                    stop=(kt == 15),
                )
            x_col = sp.tile([128, 1], BF16, tag="x_col")
            nc.scalar.activation(x_col[:], ps_x[:], AF.Relu, bias=cb)

            # ---------- partial LSTM gates ----------
            # W_hh @ h0_slice runs early (h0 is an input); W_ih @ x_slice
            # waits for x.  Separate PSUM tiles keep the accumulation
            # groups bank-sequential; summed on the way to DRAM.
            ps_gh = pp.tile([128, 32], F32, tag="ps")
            for j in range(2):
                for mt in range(16):
                    col = j * 16 + mt
                    nc.tensor.matmul(
                        ps_gh[:, col : col + 1],
                        whh[j][:, mt * 128 : (mt + 1) * 128],
                        h0s,
                        start=True,
                        stop=True,
                    )
            ghh_sb = sp.tile([128, 32], F32, tag="ghh_sb")
            nc.vector.tensor_copy(ghh_sb[:], ps_gh[:])
            ps_g = pp.tile([128, 32], F32, tag="ps")
            for j in range(2):
                for mt in range(16):
                    col = j * 16 + mt
                    nc.tensor.matmul(
                        ps_g[:, col : col + 1],
                        wih[j][:, mt * 128 : (mt + 1) * 128],
                        x_col[:],
                        start=True,
                        stop=True,
                    )
            gp_sb = sp.tile([128, 32], F32, tag="gp_sb")
            nc.vector.tensor_add(gp_sb[:], ps_g[:], ghh_sb[:])

            # ---------- AllReduce #2: sum gate partials across cores -----
            # PE-transpose the partials to [32,128] row-major so the bounce
            # write and readback are single-descriptor linear DMAs.
            ps_gt = pp.tile([32, 128], F32, tag="ps")
            nc.tensor.transpose(ps_gt[:], gp_sb[:], id128[:])
            gpr = sp.tile([32, 128], F32, tag="gpr")
            nc.vector.tensor_copy(gpr[:], ps_gt[:])
            contrib2 = dp.tile([4096], F32, tag="contrib2")
            gsum = dp.tile([4096], F32, tag="gsum")
            nc.gpsimd.dma_start(
                contrib2[:].rearrange("(r c) -> r c", c=128), gpr[:]
            )
            nc.gpsimd.collective_compute(
                "AllReduce",
                ALU.add,
                replica_groups=RG,
                ins=[contrib2[:].opt()],
                outs=[gsum[:].opt()],
            )
            g2r = sp.tile([32, 128], F32, tag="g2r")
            nc.gpsimd.dma_start(g2r[:], gsum[:].rearrange("(r c) -> r c", c=128))
            ps_gc = pp.tile([128, 32], F32, tag="ps")
            nc.tensor.transpose(ps_gc[:], g2r[:], id128[0:32, 0:32])
            gates = sp.tile([128, 32], F32, tag="gates")
            nc.vector.tensor_add(gates[:], ps_gc[:], bg)

            # ---------- LSTM elementwise (gate order i, f, g, o) ----------
            sig = sp.tile([128, 24], F32, tag="sig")
            nc.scalar.activation(sig[:, 0:8], gates[:, 0:8], AF.Sigmoid)
            nc.scalar.activation(sig[:, 8:16], gates[:, 8:16], AF.Sigmoid)
            nc.scalar.activation(sig[:, 16:24], gates[:, 24:32], AF.Sigmoid)
            gt = sp.tile([128, 8], F32, tag="gt")
            nc.scalar.activation(gt[:], gates[:, 16:24], AF.Tanh)
            cnew = sp.tile([128, 8], F32, tag="cnew")
            t1 = sp.tile([128, 8], F32, tag="t1")
            nc.vector.tensor_mul(cnew[:], sig[:, 8:16], c0c)
            nc.vector.tensor_mul(t1[:], sig[:, 0:8], gt[:])
            nc.vector.tensor_add(cnew[:], cnew[:], t1[:])
            tcn = sp.tile([128, 8], F32, tag="tcn")
            nc.scalar.activation(tcn[:], cnew[:], AF.Tanh)
            hnew = sp.tile([128, 8], F32, tag="hnew")
            nc.vector.tensor_mul(hnew[:], sig[:, 16:24], tcn[:])
            nc.sync.dma_start(o_h.ap(), hnew[:])
            nc.sync.dma_start(o_c.ap(), cnew[:])

            # ---------- output layer + log_softmax ----------
            h_bf = sp.tile([128, 8], BF16, tag="h_bf")
            nc.vector.tensor_copy(h_bf[:], hnew[:])
            ps_o = pps.tile([1, V], F32, tag="pss")
            for kt in range(8):
                nc.tensor.matmul(
                    ps_o[:],
                    h_bf[:, kt : kt + 1],
                    oW[:, kt * V : (kt + 1) * V],
                    start=(kt == 0),
                    stop=(kt == 7),
                )
            orow = sp.tile([1, V], F32, tag="orow")
            nc.vector.tensor_add(orow[:], ps_o[:], ob)
            negmx = sp.tile([1, 1], F32, tag="negmx")
            nc.vector.tensor_reduce(negmx[:], orow[:], AX.X, ALU.max, negate=True)
            erow = sp.tile([1, V], F32, tag="erow")
            sume = sp.tile([1, 1], F32, tag="sume")
            nc.scalar.activation(
                erow[:], orow[:], AF.Exp, bias=negmx[:], accum_out=sume[:]
            )
            lse = sp.tile([1, 1], F32, tag="lse")
            nc.scalar.activation(lse[:], sume[:], AF.Ln)
            lp = sp.tile([1, V], F32, tag="lp")
            nc.vector.tensor_scalar(
                lp[:], orow[:], negmx[:], lse[:], ALU.add, ALU.subtract
            )
            nc.sync.dma_start(o_lp.ap(), lp[:])

    nc.finalize()  # Bacc.finalize = compile passes (reg alloc etc) + freeze
    return nc


def prep_in_maps(inputs):
    tok = np.asarray(inputs["input_tok"]).reshape(1, 1).astype(np.int32)
    h0 = np.asarray(inputs["hidden_state"], dtype=np.float32).reshape(H)
    c0 = np.asarray(inputs["cell_state"], dtype=np.float32).reshape(H)
    enc = np.asarray(inputs["encoder_outputs"], dtype=np.float32)
    emb = _bf16(inputs["emb"])
    aW = np.asarray(inputs["attn_W"], dtype=np.float32)
    abv = np.asarray(inputs["attn_b"], dtype=np.float32)
    cWm = np.asarray(inputs["comb_W"], dtype=np.float32)
    cbv = np.asarray(inputs["comb_b"], dtype=np.float32)
    wihm = np.asarray(inputs["W_ih"], dtype=np.float32)
    whhm = np.asarray(inputs["W_hh"], dtype=np.float32)
    bgv = np.asarray(inputs["b_ih"], dtype=np.float32) + np.asarray(
        inputs["b_hh"], dtype=np.float32
    )
    oWm = np.asarray(inputs["out_W"], dtype=np.float32)
    obv = np.asarray(inputs["out_b"], dtype=np.float32)

    h0col = h0.reshape(8, 128).T
    c0col = c0.reshape(8, 128).T
    bgcol = bgv.reshape(32, 128).T
    oWp = _bf16(_pack_k(np.ascontiguousarray(oWm.T), V))
    rowf = np.zeros((1, 541), dtype=np.float32)
    rowf[0, 512:541] = obv

    in_maps = []
    for i in range(NCORES):
        aWp = _bf16(_pack_k(np.ascontiguousarray(aW[i * LSH : (i + 1) * LSH, :].T), LSH))
        encp = _bf16(_pack_k(enc[i * LSH : (i + 1) * LSH, :], H))
        cWp = _bf16(_pack_k(np.ascontiguousarray(cWm[i * HSH : (i + 1) * HSH, :].T), HSH))
        wihp = _bf16(wihm[:, i * HSH : (i + 1) * HSH].T)   # [128, 4096]
        whhp = _bf16(whhm[:, i * HSH : (i + 1) * HSH].T)
        smf = np.concatenate(
            [
                c0col,
                cbv[i * HSH : (i + 1) * HSH].reshape(128, 1),
                bgcol,
            ],
            axis=1,
        ).astype(np.float32)
        smb = _bf16(
            np.concatenate(
                [h0col, h0[i * HSH : (i + 1) * HSH].reshape(128, 1)], axis=1
            )
        )
        m = {
            "tok": tok,
            "emb": emb,
            "smf": np.ascontiguousarray(smf),
            "rowf": rowf,
            "smb": smb,
            "abb": _bf16(abv[i * LSH : (i + 1) * LSH].reshape(1, LSH)),
            "cW": cWp,
            "oW": oWp,
        }
        for j in range(4):
            m[f"aW{j}"] = np.ascontiguousarray(aWp[:, j * 2048 : (j + 1) * 2048])
        for j in range(2):
            m[f"enc{j}"] = np.ascontiguousarray(encp[:, j * 2048 : (j + 1) * 2048])
            m[f"wih{j}"] = np.ascontiguousarray(wihp[:, j * 2048 : (j + 1) * 2048])
            m[f"whh{j}"] = np.ascontiguousarray(whhp[:, j * 2048 : (j + 1) * 2048])
        in_maps.append(m)
    return in_maps


def unpack_outputs(res0):
    log_probs = np.asarray(res0["logp"], dtype=np.float32).reshape(1, V)
    h_new = np.ascontiguousarray(
        np.asarray(res0["hout"], dtype=np.float32).T
    ).reshape(1, 1, H)
    c_new = np.ascontiguousarray(
        np.asarray(res0["cout"], dtype=np.float32).T
    ).reshape(1, 1, H)
    attn_w = np.asarray(res0["awout"], dtype=np.float32).reshape(1, L)
    return (log_probs, h_new, c_new, attn_w)


def get_nc():
    if "nc" not in _CACHE:
        _CACHE["nc"] = _build_nc()
    return _CACHE["nc"]


def kernel(**inputs):
    from concourse import bass_utils

    nc = get_nc()
    in_maps = prep_in_maps(inputs)
    out = bass_utils.run_bass_kernel_spmd(nc, in_maps, core_ids=list(range(NCORES)))
    return unpack_outputs(out.results[0])


if __name__ == "__main__":
    nc = _build_nc()
    print("built ok; instructions:", sum(len(bb.instructions) for bb in nc.main_func.blocks))


# revision 20
# speedup vs baseline: 2.2722x; 1.0415x over previous
"""AttentionDecoderRNN single-step decode on 8 TRN2 NeuronCores.

Strategy (tensor-parallel matvec, 1 AllGather + 1 AllReduce):
  - attn_W sharded over output rows (512 logits/core); encoder_outputs
    sharded over the same rows, so each core computes an *unnormalized*
    attention-value partial u_i = sum_j exp(logit_ij - m_i) * enc_i[j,:]
    plus local stats (m_i, s_i) -- flash-attention style.
  - AllGather #1 carries {u_i, s_i, m_i, logits_i}; every core then
    reconstructs the exact global softmax via c_i = exp(m_i - M)/S and
    gets attn_applied plus the full attn_weights output.
  - comb_W sharded over output rows (128/core) -> local x slice (relu).
  - W_ih / W_hh sharded over input columns matching the x / h0 slice
    each core owns -> partial LSTM gates (4096).
  - AllReduce #2 (add) sums the gate partials; every core runs the LSTM
    elementwise math + output layer + log_softmax redundantly.

Performance notes (from trace analysis of the f32 v1):
  - fp32 matmuls lower to 2x(LDWEIGHTS+MATMUL) at ~430ns per 128x128
    tile -> PE consumes weights at only ~149GB/s.  v2 stores all big
    weights in bf16 (halves DMA bytes; rel err ~2.7e-3, gate is 2e-2)
    and flips the two big matvecs so the weight matrix is the *moving*
    operand: one matmul covers [128K x 512N] (16 instructions for the
    whole attention logit matvec instead of 64).
  - A single HWDGE ring sustains only ~146GB/s, so the weight streams
    are spread across the vector / tensor / gpsimd rings.  The sync
    ring is kept free for latency-critical mid-kernel DMAs (collective
    bounce traffic); the scalar ring is avoided entirely (it carries
    ~39k 4-byte event-semaphore packets and showed 15-30us completion
    latencies).
  - Gate partials go through AllReduce (CCE adds) instead of AllGather
    + on-chip sum: the strided 8-rank interleaved readback cost ~36us.
  - Activation tables (Exp/Sigmoid/Tanh/Ln) are warmed with dummy ops
    at t=0 so the 1.3us ACT_TABLE_LOADs stay off the critical path.
"""

import numpy as np

NCORES = 8
V = 29
H = 1024
L = 4096
LSH = L // NCORES    # 512 logits per core
HSH = H // NCORES    # 128 hidden per core
CTR = 1544           # contrib floats: 1024 u + 1 s + 1 m + 512 logits + 6 pad

_CACHE = {}


def _bf16(x):
    import ml_dtypes

    return np.ascontiguousarray(
        np.asarray(x, dtype=np.float32).astype(ml_dtypes.bfloat16)
    )


def _pack_k(mat, ncols):
    """[K, ncols] (K = 128*nk) -> [128, nk*ncols]; block kt lands at
    columns [kt*ncols, (kt+1)*ncols) so out[p, kt*ncols+c] = mat[kt*128+p, c]."""
    K = mat.shape[0]
    nk = K // 128
    return np.ascontiguousarray(
        mat.reshape(nk, 128, ncols).transpose(1, 0, 2).reshape(128, nk * ncols)
    )


def _build_nc():
    import concourse.bacc as bacc
    import concourse.mybir as mybir
    from concourse import tile

    F32 = mybir.dt.float32
    BF16 = mybir.dt.bfloat16
    I32 = mybir.dt.int32
    AF = mybir.ActivationFunctionType
    ALU = mybir.AluOpType
    AX = mybir.AxisListType

    nc = bacc.Bacc(
        "TRN2", target_bir_lowering=False, debug=False, num_devices=NCORES
    )

    def inp(name, shape, dt=F32):
        return nc.dram_tensor(name, shape, dt, kind="ExternalInput")

    tok_d = inp("tok", [1, 1], I32)
    emb_d = inp("emb", [V, H], BF16)
    smf_d = inp("smf", [128, 41])            # c0col | cb | bg   (f32)
    rowf_d = inp("rowf", [1, 541])           # ob pad'd row      (f32)
    smb_d = inp("smb", [128, 9], BF16)       # h0col | h0slice   (bf16)
    abb_d = inp("abb", [1, LSH], BF16)       # attn_b slice row  (bf16)
    aW_d = [inp(f"aW{j}", [128, 2048], BF16) for j in range(4)]
    enc_d = [inp(f"enc{j}", [128, 2048], BF16) for j in range(2)]
    cW_d = inp("cW", [128, 2048], BF16)
    wih_d = [inp(f"wih{j}", [128, 2048], BF16) for j in range(2)]
    whh_d = [inp(f"whh{j}", [128, 2048], BF16) for j in range(2)]
    oW_d = inp("oW", [128, 8 * V], BF16)

    o_lp = nc.dram_tensor("logp", [1, V], F32, kind="ExternalOutput")
    o_h = nc.dram_tensor("hout", [128, 8], F32, kind="ExternalOutput")
    o_c = nc.dram_tensor("cout", [128, 8], F32, kind="ExternalOutput")
    o_aw = nc.dram_tensor("awout", [NCORES, LSH], F32, kind="ExternalOutput")

    iota_d = nc.inline_tensor(
        np.arange(V, dtype=np.float32).reshape(V, 1), name="iota29"
    )
    id8_d = nc.inline_tensor(np.eye(8, dtype=np.float32), name="ident8")
    id128_d = nc.inline_tensor(np.eye(128, dtype=np.float32), name="ident128")


    RG = [list(range(NCORES))]

    with tile.TileContext(nc) as tc:
        with (
            tc.tile_pool(name="wts", bufs=1) as wp,
            tc.tile_pool(name="sml", bufs=1) as sp,
            tc.tile_pool(name="psum", bufs=4, space="PSUM") as pp,
            tc.tile_pool(name="psumu", bufs=1, space="PSUM") as ppu,
            tc.tile_pool(name="psums", bufs=1, space="PSUM") as pps,
            tc.tile_pool(name="dram", bufs=1, space="DRAM") as dp,
        ):
            # ---------- small input loads (sync ring, all tiny) ----------
            emb_sb = wp.tile([V, H], BF16, tag="emb")
            nc.sync.dma_start(emb_sb[:], emb_d.ap())
            smf = wp.tile([128, 41], F32, tag="smf")
            nc.sync.dma_start(smf[:], smf_d.ap())
            rowf = wp.tile([1, 541], F32, tag="rowf")
            nc.sync.dma_start(rowf[:], rowf_d.ap())
            smb = wp.tile([128, 9], BF16, tag="smb")
            nc.sync.dma_start(smb[:], smb_d.ap())
            abb = wp.tile([1, LSH], BF16, tag="abb")
            nc.sync.dma_start(abb[:], abb_d.ap())
            iota = wp.tile([V, 1], F32, tag="iota")
            nc.sync.dma_start(iota[:], iota_d.ap())
            id8 = wp.tile([8, 8], F32, tag="id8")
            nc.sync.dma_start(id8[:], id8_d.ap())
            id128 = wp.tile([128, 128], F32, tag="id128")
            nc.sync.dma_start(id128[:], id128_d.ap())
            tok_i = sp.tile([1, 1], I32, tag="tok_i")
            nc.sync.dma_start(tok_i[:], tok_d.ap())
            oW = wp.tile([128, 8 * V], BF16, tag="oW")
            nc.sync.dma_start(oW[:], oW_d.ap())

            c0c = smf[:, 0:8]
            cb = smf[:, 8:9]
            bg = smf[:, 9:41]
            ob = rowf[:, 512:541]
            h0c = smb[:, 0:8]
            h0s = smb[:, 8:9]

            # ---------- big weight streams, spread across rings ----------
            aW = [wp.tile([128, 2048], BF16, tag=f"aW{j}", name=f"aW{j}_sb") for j in range(4)]
            encs = [wp.tile([128, 2048], BF16, tag=f"enc{j}", name=f"enc{j}_sb") for j in range(2)]
            cW = wp.tile([128, 2048], BF16, tag="cW")
            whh = [wp.tile([128, 2048], BF16, tag=f"whh{j}", name=f"whh{j}_sb") for j in range(2)]
            wih = [wp.tile([128, 2048], BF16, tag=f"wih{j}", name=f"wih{j}_sb") for j in range(2)]
            # gpsimd SWDGE ring (measured ~270GB/s, clean completions):
            # the dependency-chain weights, in consumption order.  The
            # sync HWDGE ring carries the event-semaphore flood, which
            # delays completion semaphores by up to ~33us -- so only the
            # LSTM weights (whose consumers wait on data sems and run
            # late anyway) plus the small preloads go there.
            for j in range(2):
                nc.gpsimd.dma_start(aW[j][:], aW_d[j].ap())
            for j in range(2, 4):
                nc.sync.dma_start(aW[j][:], aW_d[j].ap())
            for j in range(2):
                nc.gpsimd.dma_start(encs[j][:], enc_d[j].ap())
            nc.gpsimd.dma_start(cW[:], cW_d.ap())
            for j in range(2):
                nc.sync.dma_start(whh[j][:], whh_d[j].ap())
            for j in range(2):
                nc.sync.dma_start(wih[j][:], wih_d[j].ap())

            # ---------- constants + activation-table warmup ----------
            ones29 = sp.tile([1, V], F32, tag="ones29")
            nc.vector.memset(ones29[:], 1.0)
            ones8r = sp.tile([1, 8], F32, tag="ones8r")
            nc.vector.memset(ones8r[:], 1.0)
            ones8c = sp.tile([8, 1], F32, tag="ones8c")
            nc.vector.memset(ones8c[:], 1.0)
            one_bf = sp.tile([1, 1], BF16, tag="one_bf")
            nc.vector.memset(one_bf[:], 1.0)
            zpad = sp.tile([1, 6], F32, tag="zpad")
            nc.vector.memset(zpad[:], 0.0)
            warm = sp.tile([1, 4], F32, tag="warm")
            nc.vector.memset(warm[:], 0.5)
            for fn in (AF.Exp, AF.Sigmoid, AF.Tanh, AF.Ln):
                nc.scalar.activation(warm[:], warm[:], fn)

            # ---------- embedded = emb[tok] via on-device one-hot ----------
            tok_f = sp.tile([1, 1], F32, tag="tok_f")
            nc.vector.tensor_copy(tok_f[:], tok_i[:])
            ps_tb = pps.tile([V, 1], F32, tag="pss")
            nc.tensor.matmul(ps_tb[:], ones29[:], tok_f[:], start=True, stop=True)
            tokb = sp.tile([V, 1], F32, tag="tokb")
            nc.vector.tensor_copy(tokb[:], ps_tb[:])
            onehot = sp.tile([V, 1], F32, tag="onehot")
            nc.vector.tensor_tensor(onehot[:], iota[:], tokb[:], ALU.is_equal)
            ohb = sp.tile([V, 1], BF16, tag="ohb")
            nc.vector.tensor_copy(ohb[:], onehot[:])
            ps_emb = pp.tile([128, 8], F32, tag="ps")
            for ht in range(8):
                nc.tensor.matmul(
                    ps_emb[:, ht : ht + 1],
                    emb_sb[:, ht * 128 : (ht + 1) * 128],
                    ohb[:],
                    start=True,
                    stop=True,
                )
            cat2 = sp.tile([128, 16], BF16, tag="cat2")
            nc.vector.tensor_copy(cat2[:, 0:8], ps_emb[:])

            # ---------- attention logits (row form, weights moving) ------
            # logits[l] = sum_d cat1[d] * attn_W[l, d] + attn_b[l]
            ps_lg = ppu.tile([1, LSH], F32, tag="pslg")
            for kt in range(16):
                vec = cat2[:, kt : kt + 1] if kt < 8 else h0c[:, kt - 8 : kt - 7]
                nc.tensor.matmul(
                    ps_lg[:],
                    vec,
                    aW[kt // 4][:, (kt % 4) * 512 : (kt % 4 + 1) * 512],
                    start=(kt == 0),
                    stop=False,
                )
            nc.tensor.matmul(ps_lg[:], one_bf[:], abb[:], start=False, stop=True)

            # local softmax stats on the row
            negm = sp.tile([1, 1], F32, tag="negm")
            nc.vector.tensor_reduce(negm[:], ps_lg[:], AX.X, ALU.max, negate=True)
            expr = sp.tile([1, LSH], F32, tag="expr")
            s_row = sp.tile([1, 1], F32, tag="s_row")
            nc.scalar.activation(
                expr[:], ps_lg[:], AF.Exp, bias=negm[:], accum_out=s_row[:]
            )
            sm = sp.tile([1, 2], F32, tag="sm")
            nc.vector.tensor_copy(sm[:, 0:1], s_row[:])
            nc.vector.tensor_scalar_mul(sm[:, 1:2], negm[:], -1.0)

            # transpose exp weights row -> column [128, 4] (f32: PSUM
            # writes must stay 4-byte aligned; cast to bf16 on the copy out)
            ps_tc = pp.tile([128, 4], F32, tag="ps")
            for j in range(4):
                nc.tensor.transpose(
                    ps_tc[:, j : j + 1], expr[:, j * 128 : (j + 1) * 128],
                    ones29[:, 0:1]
                )
            expc = sp.tile([128, 4], BF16, tag="expc")
            nc.vector.tensor_copy(expc[:], ps_tc[:])

            # ---------- u = expw @ enc_shard (row form) ----------
            ps_u = ppu.tile([1, H], F32, tag="psu")
            for nt in range(2):
                for kt in range(4):
                    nc.tensor.matmul(
                        ps_u[:, nt * 512 : (nt + 1) * 512],
                        expc[:, kt : kt + 1],
                        encs[kt // 2][:, (kt % 2) * 1024 + nt * 512 : (kt % 2) * 1024 + nt * 512 + 512],
                        start=(kt == 0),
                        stop=(kt == 3),
                    )

            # ---------- AllGather #1: {u, s, m, logits} ----------
            # Assemble the whole 1544-float payload in one SBUF row so the
            # bounce write is a single-descriptor DMA (SWDGE descriptor
            # emission costs ~0.7us each; a strided write would need 32).
            crow = sp.tile([1, CTR], F32, tag="crow")
            nc.vector.memset(crow[:, 1538:1544], 0.0)
            nc.vector.tensor_copy(crow[:, 0:512], ps_u[:, 0:512])
            nc.scalar.copy(crow[:, 512:1024], ps_u[:, 512:1024])
            nc.vector.tensor_copy(crow[:, 1024:1026], sm[:])
            nc.vector.tensor_copy(crow[:, 1026:1538], ps_lg[:])
            contrib = dp.tile([CTR], F32, tag="contrib")
            gath1 = dp.tile([NCORES, CTR], F32, tag="gath1")
            nc.gpsimd.dma_start(
                contrib[:].rearrange("(a b) -> a b", a=1), crow[:]
            )
            nc.gpsimd.collective_compute(
                "AllGather",
                ALU.bypass,
                replica_groups=RG,
                ins=[contrib[:].opt()],
                outs=[gath1[:].opt()],
            )
            g1 = sp.tile([NCORES, CTR], F32, tag="g1")
            nc.gpsimd.dma_start(g1[:], gath1[:])

            # ---------- softmax rescale coefficients c_i = exp(m_i-M)/S --
            m_col = g1[0:8, 1025:1026]
            s_col = g1[0:8, 1024:1025]
            ps_t8 = pps.tile([1, 8], F32, tag="pss")
            nc.tensor.transpose(ps_t8[:], m_col, id8[:])
            mrow = sp.tile([1, 8], F32, tag="mrow")
            nc.vector.tensor_copy(mrow[:], ps_t8[:])
            negM = sp.tile([1, 1], F32, tag="negM")
            nc.vector.tensor_reduce(negM[:], mrow[:], AX.X, ALU.max, negate=True)
            ps_nM = pps.tile([8, 1], F32, tag="pss")
            nc.tensor.matmul(ps_nM[:], ones8r[:], negM[:], start=True, stop=True)
            negMb = sp.tile([8, 1], F32, tag="negMb")
            nc.vector.tensor_copy(negMb[:], ps_nM[:])
            d_col = sp.tile([8, 1], F32, tag="d_col")
            nc.scalar.activation(d_col[:], m_col, AF.Exp, bias=negMb[:])
            sd_col = sp.tile([8, 1], F32, tag="sd_col")
            nc.vector.tensor_mul(sd_col[:], d_col[:], s_col)
            ps_S = pps.tile([1, 1], F32, tag="pss")
            nc.tensor.matmul(ps_S[:], sd_col[:], ones8c[:], start=True, stop=True)
            S_sb = sp.tile([1, 1], F32, tag="S_sb")
            nc.vector.tensor_copy(S_sb[:], ps_S[:])
            rS = sp.tile([1, 1], F32, tag="rS")
            nc.vector.reciprocal(rS[:], S_sb[:])
            ps_rS = pps.tile([8, 1], F32, tag="pss")
            nc.tensor.matmul(ps_rS[:], ones8r[:], rS[:], start=True, stop=True)
            rSb = sp.tile([8, 1], F32, tag="rSb")
            nc.vector.tensor_copy(rSb[:], ps_rS[:])
            c_col = sp.tile([8, 1], F32, tag="c_col")
            nc.vector.tensor_mul(c_col[:], d_col[:], rSb[:])

            # ---------- attn_applied = sum_i c_i * u_i ----------
            ps_aa = pp.tile([128, 8], F32, tag="ps")
            for ht in range(8):
                nc.tensor.matmul(
                    ps_aa[:, ht : ht + 1],
                    g1[0:8, ht * 128 : (ht + 1) * 128],
                    c_col[:],
                    start=True,
                    stop=True,
                )
            nc.vector.tensor_copy(cat2[:, 8:16], ps_aa[:])

            # ---------- attn_weights output (full, exact softmax) --------
            negm8 = sp.tile([8, 1], F32, tag="negm8")
            nc.vector.tensor_scalar_mul(negm8[:], m_col, -1.0)
            aw = sp.tile([8, LSH], F32, tag="aw")
            nc.scalar.activation(aw[:], g1[0:8, 1026:1538], AF.Exp, bias=negm8[:])
            nc.vector.tensor_scalar_mul(aw[:], aw[:], c_col[:])
            nc.sync.dma_start(o_aw.ap(), aw[:])

            # ---------- x slice = relu(comb_W_rows @ cat2 + b) ----------
            ps_x = pp.tile([128, 1], F32, tag="ps")
            for kt in range(16):
                nc.tensor.matmul(
                    ps_x[:],
                    cW[:, kt * 128 : (kt + 1) * 128],
                    cat2[:, kt : kt + 1],
                    start=(kt == 0),
                    stop=(kt == 15),
                )
            x_col = sp.tile([128, 1], BF16, tag="x_col")
            nc.scalar.activation(x_col[:], ps_x[:], AF.Relu, bias=cb)

            # ---------- partial LSTM gates ----------
            # W_hh @ h0_slice runs early (h0 is an input); W_ih @ x_slice
            # waits for x.  Separate PSUM tiles keep the accumulation
            # groups bank-sequential; summed on the way to DRAM.
            ps_gh = pp.tile([128, 32], F32, tag="ps")
            for j in range(2):
                for mt in range(16):
                    col = j * 16 + mt
                    nc.tensor.matmul(
                        ps_gh[:, col : col + 1],
                        whh[j][:, mt * 128 : (mt + 1) * 128],
                        h0s,
                        start=True,
                        stop=True,
                    )
            ghh_sb = sp.tile([128, 32], F32, tag="ghh_sb")
            nc.vector.tensor_copy(ghh_sb[:], ps_gh[:])
            ps_g = pp.tile([128, 32], F32, tag="ps")
            for j in range(2):
                for mt in range(16):
                    col = j * 16 + mt
                    nc.tensor.matmul(
                        ps_g[:, col : col + 1],
                        wih[j][:, mt * 128 : (mt + 1) * 128],
                        x_col[:],
                        start=True,
                        stop=True,
                    )
            gp_sb = sp.tile([128, 32], F32, tag="gp_sb")
            nc.vector.tensor_add(gp_sb[:], ps_g[:], ghh_sb[:])

            # ---------- AllReduce #2: sum gate partials across cores -----
            # PE-transpose the partials to [32,128] row-major so the bounce
            # write and readback are single-descriptor linear DMAs.
            ps_gt = pp.tile([32, 128], F32, tag="ps")
            nc.tensor.transpose(ps_gt[:], gp_sb[:], id128[:])
            gpr = sp.tile([32, 128], F32, tag="gpr")
            nc.vector.tensor_copy(gpr[:], ps_gt[:])
            contrib2 = dp.tile([4096], F32, tag="contrib2")
            gsum = dp.tile([4096], F32, tag="gsum")
            nc.gpsimd.dma_start(
                contrib2[:].rearrange("(r c) -> r c", c=128), gpr[:]
            )
            nc.gpsimd.collective_compute(
                "AllReduce",
                ALU.add,
                replica_groups=RG,
                ins=[contrib2[:].opt()],
                outs=[gsum[:].opt()],
            )
            g2r = sp.tile([32, 128], F32, tag="g2r")
            nc.gpsimd.dma_start(g2r[:], gsum[:].rearrange("(r c) -> r c", c=128))
            ps_gc = pp.tile([128, 32], F32, tag="ps")
            nc.tensor.transpose(ps_gc[:], g2r[:], id128[0:32, 0:32])
            gates = sp.tile([128, 32], F32, tag="gates")
            nc.vector.tensor_add(gates[:], ps_gc[:], bg)

            # ---------- LSTM elementwise (gate order i, f, g, o) ----------
            sig = sp.tile([128, 24], F32, tag="sig")
            nc.scalar.activation(sig[:, 0:16], gates[:, 0:16], AF.Sigmoid)
            nc.scalar.activation(sig[:, 16:24], gates[:, 24:32], AF.Sigmoid)
            gt = sp.tile([128, 8], F32, tag="gt")
            nc.scalar.activation(gt[:], gates[:, 16:24], AF.Tanh)
            cnew = sp.tile([128, 8], F32, tag="cnew")
            t1 = sp.tile([128, 8], F32, tag="t1")
            nc.vector.tensor_mul(cnew[:], sig[:, 8:16], c0c)
            nc.vector.tensor_mul(t1[:], sig[:, 0:8], gt[:])
            nc.vector.tensor_add(cnew[:], cnew[:], t1[:])
            tcn = sp.tile([128, 8], F32, tag="tcn")
            nc.scalar.activation(tcn[:], cnew[:], AF.Tanh)
            hnew = sp.tile([128, 8], F32, tag="hnew")
            nc.vector.tensor_mul(hnew[:], sig[:, 16:24], tcn[:])
            nc.sync.dma_start(o_h.ap(), hnew[:])
            nc.sync.dma_start(o_c.ap(), cnew[:])

            # ---------- output layer + log_softmax ----------
            h_bf = sp.tile([128, 8], BF16, tag="h_bf")
            nc.vector.tensor_copy(h_bf[:], hnew[:])
            ps_o = pps.tile([1, V], F32, tag="pss")
            for kt in range(8):
                nc.tensor.matmul(
                    ps_o[:],
                    h_bf[:, kt : kt + 1],
                    oW[:, kt * V : (kt + 1) * V],
                    start=(kt == 0),
                    stop=(kt == 7),
                )
            orow = sp.tile([1, V], F32, tag="orow")
            nc.vector.tensor_add(orow[:], ps_o[:], ob)
            negmx = sp.tile([1, 1], F32, tag="negmx")
            nc.vector.tensor_reduce(negmx[:], orow[:], AX.X, ALU.max, negate=True)
            erow = sp.tile([1, V], F32, tag="erow")
            sume = sp.tile([1, 1], F32, tag="sume")
            nc.scalar.activation(
                erow[:], orow[:], AF.Exp, bias=negmx[:], accum_out=sume[:]
            )
            lse = sp.tile([1, 1], F32, tag="lse")
            nc.scalar.activation(lse[:], sume[:], AF.Ln)
            lp = sp.tile([1, V], F32, tag="lp")
            nc.vector.tensor_scalar(
                lp[:], orow[:], negmx[:], lse[:], ALU.add, ALU.subtract
            )
            nc.sync.dma_start(o_lp.ap(), lp[:])

    nc.finalize()  # Bacc.finalize = compile passes (reg alloc etc) + freeze
    return nc


def prep_in_maps(inputs):
    tok = np.asarray(inputs["input_tok"]).reshape(1, 1).astype(np.int32)
    h0 = np.asarray(inputs["hidden_state"], dtype=np.float32).reshape(H)
    c0 = np.asarray(inputs["cell_state"], dtype=np.float32).reshape(H)
    enc = np.asarray(inputs["encoder_outputs"], dtype=np.float32)
    emb = _bf16(inputs["emb"])
    aW = np.asarray(inputs["attn_W"], dtype=np.float32)
    abv = np.asarray(inputs["attn_b"], dtype=np.float32)
    cWm = np.asarray(inputs["comb_W"], dtype=np.float32)
    cbv = np.asarray(inputs["comb_b"], dtype=np.float32)
    wihm = np.asarray(inputs["W_ih"], dtype=np.float32)
    whhm = np.asarray(inputs["W_hh"], dtype=np.float32)
    bgv = np.asarray(inputs["b_ih"], dtype=np.float32) + np.asarray(
        inputs["b_hh"], dtype=np.float32
    )
    oWm = np.asarray(inputs["out_W"], dtype=np.float32)
    obv = np.asarray(inputs["out_b"], dtype=np.float32)

    h0col = h0.reshape(8, 128).T
    c0col = c0.reshape(8, 128).T
    bgcol = bgv.reshape(32, 128).T
    oWp = _bf16(_pack_k(np.ascontiguousarray(oWm.T), V))
    rowf = np.zeros((1, 541), dtype=np.float32)
    rowf[0, 512:541] = obv

    in_maps = []
    for i in range(NCORES):
        aWp = _bf16(_pack_k(np.ascontiguousarray(aW[i * LSH : (i + 1) * LSH, :].T), LSH))
        encp = _bf16(_pack_k(enc[i * LSH : (i + 1) * LSH, :], H))
        cWp = _bf16(_pack_k(np.ascontiguousarray(cWm[i * HSH : (i + 1) * HSH, :].T), HSH))
        wihp = _bf16(wihm[:, i * HSH : (i + 1) * HSH].T)   # [128, 4096]
        whhp = _bf16(whhm[:, i * HSH : (i + 1) * HSH].T)
        smf = np.concatenate(
            [
                c0col,
                cbv[i * HSH : (i + 1) * HSH].reshape(128, 1),
                bgcol,
            ],
            axis=1,
        ).astype(np.float32)
        smb = _bf16(
            np.concatenate(
                [h0col, h0[i * HSH : (i + 1) * HSH].reshape(128, 1)], axis=1
            )
        )
        m = {
            "tok": tok,
            "emb": emb,
            "smf": np.ascontiguousarray(smf),
            "rowf": rowf,
            "smb": smb,
            "abb": _bf16(abv[i * LSH : (i + 1) * LSH].reshape(1, LSH)),
            "cW": cWp,
            "oW": oWp,
        }
        for j in range(4):
            m[f"aW{j}"] = np.ascontiguousarray(aWp[:, j * 2048 : (j + 1) * 2048])
        for j in range(2):
            m[f"enc{j}"] = np.ascontiguousarray(encp[:, j * 2048 : (j + 1) * 2048])
            m[f"wih{j}"] = np.ascontiguousarray(wihp[:, j * 2048 : (j + 1) * 2048])
            m[f"whh{j}"] = np.ascontiguousarray(whhp[:, j * 2048 : (j + 1) * 2048])
        in_maps.append(m)
    return in_maps


def unpack_outputs(res0):
    log_probs = np.asarray(res0["logp"], dtype=np.float32).reshape(1, V)
    h_new = np.ascontiguousarray(
        np.asarray(res0["hout"], dtype=np.float32).T
    ).reshape(1, 1, H)
    c_new = np.ascontiguousarray(
        np.asarray(res0["cout"], dtype=np.float32).T
    ).reshape(1, 1, H)
    attn_w = np.asarray(res0["awout"], dtype=np.float32).reshape(1, L)
    return (log_probs, h_new, c_new, attn_w)


def get_nc():
    if "nc" not in _CACHE:
        _CACHE["nc"] = _build_nc()
    return _CACHE["nc"]


def kernel(**inputs):
    from concourse import bass_utils

    nc = get_nc()
    in_maps = prep_in_maps(inputs)
    out = bass_utils.run_bass_kernel_spmd(nc, in_maps, core_ids=list(range(NCORES)))
    return unpack_outputs(out.results[0])


if __name__ == "__main__":
    nc = _build_nc()
    print("built ok; instructions:", sum(len(bb.instructions) for bb in nc.main_func.blocks))
